# revision 3
# baseline (speedup 1.0000x reference)
"""CrossEncoderGNN (2x GIN layer + sum-pool + MLP + sigmoid) on 8 trn2 NeuronCores.

Strategy
--------
Math: GIN layer  h' = (h + A h) @ W + b  ==  (I + A) (h @ W) + b   (A acts on
rows, W on columns, so they commute).  Per layer:
  phase A: y = h @ W computed on each core for its 2500-node shard (dense
           matmul, xbar-transpose DMA provides h^T tiles as lhsT).
  AllGather: y shards (f16) -> full padded table [8*2560, 512] on every core.
  phase B: per dst-tile of 128 nodes, dma_gather the y rows of all incident
           edges (dst-sorted, self-loops included) and segment-sum them with a
           one-hot [128e x 128d] matmul into PSUM; add bias.
Pooling (graph segment-sum) is one more one-hot matmul accumulated over the
core's 20 node tiles; partial pooled [64,512] is AllReduced, and the tiny
classifier MLP + sigmoid runs replicated on every core.

Sharding: nodes (and their incident in-edges) are split 8 ways by contiguous
dst ranges: core c owns nodes [2500c, 2500c+2500), padded to 2560 rows so
every core has 20 uniform tiles of 128.
"""

import sys

for _p in ("/opt/trn_rl_repo", "/root/.axon_site/_ro/trn_rl_repo"):
    if _p not in sys.path:
        sys.path.insert(0, _p)

import os
import numpy as np
import ml_dtypes

import concourse.bass as bass
import concourse.bacc as bacc
import concourse.tile as tile
from concourse import mybir
from concourse.bass_utils import run_bass_kernel_spmd
from concourse.masks import make_identity

F16 = np.float16

N_NODES = 20000
N_EDGES = 320000
D = 512
N_GRAPHS = 64
N_CORES = 8
ROWS = N_NODES // N_CORES          # 2500 real rows per core
P = 128
TILES = (ROWS + P - 1) // P        # 20
PAD_ROWS = TILES * P               # 2560 padded rows per core
FULL_PAD = PAD_ROWS * N_CORES      # 20480
KCH = D // P                       # 4 contraction chunks of 128

LAST_EXEC_NS = None
LAST_RESULTS = None

_prog_cache = {}


def _gather_split(k_max):
    """Split k_max chunks into balanced calls of <=6 chunks each."""
    n_calls = max(1, (k_max + 5) // 6)
    base = k_max // n_calls
    rem = k_max - base * n_calls
    return [base + (1 if i < rem else 0) for i in range(n_calls)]


def _build_program(k_max):
    sizes = _gather_split(k_max)
    kg_max = max(sizes)
    f32 = mybir.dt.float32
    f16 = mybir.dt.float16
    i16 = mybir.dt.int16

    nc = bacc.Bacc("TRN2", debug=False, num_devices=N_CORES)

    # ---- I/O ----
    x_sh = nc.dram_tensor("x_sh", [PAD_ROWS, D], f16, kind="ExternalInput")
    idx_all = nc.dram_tensor("idx_all", [P, TILES * k_max * 8], i16, kind="ExternalInput")
    s_all = nc.dram_tensor("s_all", [P, TILES * k_max * P], f16, kind="ExternalInput")
    p_all = nc.dram_tensor("p_all", [P, TILES * N_GRAPHS], f16, kind="ExternalInput")
    w1_in = nc.dram_tensor("w1", [P, KCH * D], f16, kind="ExternalInput")
    w2_in = nc.dram_tensor("w2", [P, KCH * D], f16, kind="ExternalInput")
    b1_in = nc.dram_tensor("b1b", [P, D], f32, kind="ExternalInput")
    b2_in = nc.dram_tensor("b2b", [P, D], f32, kind="ExternalInput")
    wc1_in = nc.dram_tensor("wc1", [P, KCH * 2 * P], f32, kind="ExternalInput")
    bc1_in = nc.dram_tensor("bc1", [P, 2], f32, kind="ExternalInput")
    wc2_in = nc.dram_tensor("wc2", [P, 2], f32, kind="ExternalInput")
    bc2_in = nc.dram_tensor("bc2", [1, 1], f32, kind="ExternalInput")
    scores = nc.dram_tensor("scores", [1, N_GRAPHS], f32, kind="ExternalOutput")

    # ---- internal DRAM ----
    y1_sh = nc.dram_tensor("y1_sh", [PAD_ROWS, D], f16)
    y2_sh = nc.dram_tensor("y2_sh", [PAD_ROWS, D], f16)
    h1_sh = nc.dram_tensor("h1_sh", [PAD_ROWS, D], f16)
    y1_full = nc.dram_tensor("y1_full", [FULL_PAD, D], f16, addr_space="Shared")
    y2_full = nc.dram_tensor("y2_full", [FULL_PAD, D], f16, addr_space="Shared")
    pool_in = nc.dram_tensor("pool_in", [N_GRAPHS, D], f32)
    pool_out = nc.dram_tensor("pool_out", [N_GRAPHS, D], f32, addr_space="Shared")

    rg = [list(range(N_CORES))]

    with tile.TileContext(nc) as tc:
        with (
            tc.tile_pool(name="const", bufs=1) as const,
            tc.tile_pool(name="xT", bufs=1) as xT_pool,
            tc.tile_pool(name="gbuf", bufs=3) as gpool,
            tc.tile_pool(name="stage", bufs=2) as stage_pool,
            tc.tile_pool(name="h2p", bufs=2) as h2_pool,
            tc.tile_pool(name="mlp", bufs=1) as mlp_pool,
            tc.tile_pool(name="psA", bufs=2, space="PSUM") as psA,
            tc.tile_pool(name="psPool", bufs=1, space="PSUM") as psPool,
            tc.tile_pool(name="psMLP", bufs=2, space="PSUM") as psMLP,
        ):
            # ---- resident constants ----
            idx_sb = const.tile([P, TILES * k_max * 8], i16)
            nc.sync.dma_start(out=idx_sb[:], in_=idx_all[:])
            s_flat = const.tile([P, TILES * k_max * P], f16)
            nc.sync.dma_start(out=s_flat[:], in_=s_all[:])
            s_sb = s_flat[:].rearrange("p (c d) -> p c d", d=P)
            p_flat = const.tile([P, TILES * N_GRAPHS], f16)
            nc.sync.dma_start(out=p_flat[:], in_=p_all[:])
            p_sb = p_flat[:].rearrange("p (t g) -> p t g", g=N_GRAPHS)
            w_sb = []
            for w_in in (w1_in, w2_in):
                wt = const.tile([P, KCH * D], f16)
                nc.sync.dma_start(out=wt[:], in_=w_in[:])
                w_sb.append(wt[:].rearrange("p (j d) -> p j d", d=D))
            b_sb = []
            for b_in in (b1_in, b2_in):
                bt = const.tile([P, D], f32)
                nc.sync.dma_start(out=bt[:], in_=b_in[:])
                b_sb.append(bt)
            wc1_sb = const.tile([P, KCH * 2 * P], f32)
            nc.sync.dma_start(out=wc1_sb[:], in_=wc1_in[:])
            wc1_v = wc1_sb[:].rearrange("p (j c m) -> p j c m", c=2, m=P)
            bc1_sb = const.tile([P, 2], f32)
            nc.sync.dma_start(out=bc1_sb[:], in_=bc1_in[:])
            wc2_sb = const.tile([P, 2], f32)
            nc.sync.dma_start(out=wc2_sb[:], in_=wc2_in[:])
            bc2_sb = const.tile([1, 1], f32)
            nc.sync.dma_start(out=bc2_sb[:], in_=bc2_in[:])
            ident = const.tile([P, P], f32)
            make_identity(nc, ident[:])

            def phase_a(h_dram, w_view, y_dram):
                """y = h @ W for this core's 20 row tiles."""
                xT = xT_pool.tile([P, KCH, PAD_ROWS], f16, tag="xT")
                for j in range(KCH):
                    nc.sync.dma_start(
                        out=xT[:, j, :],
                        in_=h_dram[:, j * P : (j + 1) * P],
                        transpose=True,
                    )
                y_view = y_dram.ap().rearrange("(t p) d -> p t d", p=P)
                for grp in range(TILES // 4):
                    st = stage_pool.tile([P, 4, D], f16, tag="stage")
                    for tt in range(4):
                        t = grp * 4 + tt
                        ps = psA.tile([P, D], f32, tag="psA")
                        for j in range(KCH):
                            nc.tensor.matmul(
                                out=ps[:],
                                lhsT=xT[:, j, t * P : (t + 1) * P],
                                rhs=w_view[:, j, :],
                                start=(j == 0),
                                stop=(j == KCH - 1),
                            )
                        nc.vector.tensor_copy(out=st[:, tt, :], in_=ps[:])
                    nc.sync.dma_start(
                        out=y_view[:, grp * 4 : grp * 4 + 4, :], in_=st[:]
                    )

            def phase_b_tiles(y_full):
                """Yield (t, agg_psum) for each dst tile."""
                for t in range(TILES):
                    gs = []
                    off = 0
                    for sz in sizes:
                        g = gpool.tile([P, kg_max, D], f16, tag="g")
                        col0 = (t * k_max + off) * 8
                        nc.gpsimd.dma_gather(
                            out_ap=g[:, :sz, :],
                            in_ap=y_full[:],
                            idxs_ap=idx_sb[:, col0 : col0 + sz * 8],
                            num_idxs=sz * P,
                            num_idxs_reg=sz * P,
                            elem_size=D,
                        )
                        gs.append((g, off, sz))
                        off += sz
                    ps = psA.tile([P, D], f32, tag="psA")
                    ki = 0
                    for g, off, sz in gs:
                        for k in range(sz):
                            nc.tensor.matmul(
                                out=ps[:],
                                lhsT=s_sb[:, t * k_max + off + k, :],
                                rhs=g[:, k, :],
                                start=(ki == 0),
                                stop=(ki == k_max - 1),
                            )
                            ki += 1
                    yield t, ps

            # ---- layer 1 ----
            phase_a(x_sh, w_sb[0], y1_sh)
            nc.gpsimd.collective_compute(
                "AllGather", mybir.AluOpType.bypass, replica_groups=rg,
                ins=[y1_sh[:]], outs=[y1_full[:]],
            )
            h1_view = h1_sh.ap().rearrange("(t p) d -> p t d", p=P)
            st = None
            for t, ps in phase_b_tiles(y1_full):
                if t % 4 == 0:
                    st = stage_pool.tile([P, 4, D], f16, tag="stage")
                nc.vector.tensor_add(out=st[:, t % 4, :], in0=ps[:], in1=b_sb[0][:])
                if t % 4 == 3:
                    g0 = t - 3
                    nc.sync.dma_start(
                        out=h1_view[:, g0 : g0 + 4, :], in_=st[:]
                    )

            # ---- layer 2 ----
            phase_a(h1_sh, w_sb[1], y2_sh)
            nc.gpsimd.collective_compute(
                "AllGather", mybir.AluOpType.bypass, replica_groups=rg,
                ins=[y2_sh[:]], outs=[y2_full[:]],
            )
            pool_ps = psPool.tile([N_GRAPHS, D], f32)
            for t, ps in phase_b_tiles(y2_full):
                h2 = h2_pool.tile([P, D], f16, tag="h2")
                nc.vector.tensor_add(out=h2[:], in0=ps[:], in1=b_sb[1][:])
                nc.tensor.matmul(
                    out=pool_ps[:],
                    lhsT=p_sb[:, t, :],
                    rhs=h2[:],
                    start=(t == 0),
                    stop=(t == TILES - 1),
                    skip_group_check=True,
                )

            # ---- pooled AllReduce ----
            pool_sb = mlp_pool.tile([N_GRAPHS, D], f32)
            nc.vector.tensor_copy(out=pool_sb[:], in_=pool_ps[:])
            nc.sync.dma_start(out=pool_in[:], in_=pool_sb[:])
            nc.gpsimd.collective_compute(
                "AllReduce", mybir.AluOpType.add, replica_groups=rg,
                ins=[pool_in[:]], outs=[pool_out[:]],
            )

            # ---- classifier MLP (replicated, all f32) ----
            pooled = mlp_pool.tile([N_GRAPHS, D], f32)
            nc.sync.dma_start(out=pooled[:], in_=pool_out[:])
            pooledT = mlp_pool.tile([P, KCH, N_GRAPHS], f32)
            for j in range(KCH):
                ps_t = psMLP.tile([P, N_GRAPHS], f32, tag="psT")
                nc.tensor.transpose(
                    out=ps_t[:],
                    in_=pooled[:, j * P : (j + 1) * P],
                    identity=ident[0:N_GRAPHS, 0:N_GRAPHS],
                )
                nc.vector.tensor_copy(out=pooledT[:, j, :], in_=ps_t[:])
            zT = mlp_pool.tile([P, 2, N_GRAPHS], f32)
            for c2 in range(2):
                ps_z = psMLP.tile([P, N_GRAPHS], f32, tag="psT")
                for j in range(KCH):
                    nc.tensor.matmul(
                        out=ps_z[:],
                        lhsT=wc1_v[:, j, c2, :],
                        rhs=pooledT[:, j, :],
                        start=(j == 0),
                        stop=(j == KCH - 1),
                    )
                nc.scalar.activation(
                    out=zT[:, c2, :], in_=ps_z[:],
                    func=mybir.ActivationFunctionType.Relu,
                    bias=bc1_sb[:, c2 : c2 + 1],
                )
            ps_s = psMLP.tile([1, N_GRAPHS], f32, tag="psS")
            for c2 in range(2):
                nc.tensor.matmul(
                    out=ps_s[:],
                    lhsT=wc2_sb[:, c2 : c2 + 1],
                    rhs=zT[:, c2, :],
                    start=(c2 == 0),
                    stop=(c2 == 1),
                )
            score_sb = mlp_pool.tile([1, N_GRAPHS], f32)
            nc.scalar.activation(
                out=score_sb[:], in_=ps_s[:],
                func=mybir.ActivationFunctionType.Sigmoid,
                bias=bc2_sb[0:1, 0:1],
            )
            nc.sync.dma_start(out=scores[:], in_=score_sb[:])

    nc.finalize()
    return nc


def _wrap_idx(block):
    """[n] -> [16, n/16] wrapped: element i at [i%16, i//16]."""
    n = block.shape[0]
    return block.reshape(n // 16, 16).T


def _prep_inputs(joint_x, joint_edge_index, joint_batch,
                 W_g1, b_g1, W_g2, b_g2, W_c1, b_c1, W_c2, b_c2):
    x = np.asarray(joint_x, np.float32)
    ei = np.asarray(joint_edge_index).astype(np.int64)
    batch = np.asarray(joint_batch).astype(np.int64)
    src, dst = ei[0], ei[1]

    # self loops
    all_nodes = np.arange(N_NODES, dtype=np.int64)
    src = np.concatenate([src, all_nodes])
    dst = np.concatenate([dst, all_nodes])

    core_of = dst // ROWS
    local_dst = dst - core_of * ROWS
    psrc = (src // ROWS) * PAD_ROWS + (src % ROWS)   # padded-id space

    # per-(core, tile) edge counts to find k_max
    tile_of = local_dst // P
    key = core_of * TILES + tile_of
    counts = np.bincount(key, minlength=N_CORES * TILES)
    k_max = int((counts.max() + P - 1) // P)

    per_core = []
    for c in range(N_CORES):
        m = core_of == c
        ld = local_dst[m]
        ps = psrc[m]
        t = ld // P
        order = np.argsort(t, kind="stable")
        ld, ps, t = ld[order], ps[order], t[order]
        cnt = np.bincount(t, minlength=TILES)
        # flat padded edge arrays [TILES, k_max*128]
        idx_flat = np.zeros((TILES, k_max * P), np.int16)
        slot_flat = np.full((TILES, k_max * P), -1, np.int64)
        starts = np.concatenate([[0], np.cumsum(cnt)])
        pos_in_tile = np.arange(len(ld)) - starts[t]
        idx_flat[t, pos_in_tile] = ps.astype(np.int16)
        slot_flat[t, pos_in_tile] = ld % P
        per_core.append((idx_flat, slot_flat))

    sizes = _gather_split(k_max)

    in_maps = []
    w1_pack = np.ascontiguousarray(
        W_g1.astype(F16).reshape(KCH, P, D).transpose(1, 0, 2).reshape(P, KCH * D))
    w2_pack = np.ascontiguousarray(
        W_g2.astype(F16).reshape(KCH, P, D).transpose(1, 0, 2).reshape(P, KCH * D))
    b1_pack = np.ascontiguousarray(np.broadcast_to(
        np.asarray(b_g1, np.float32), (P, D)))
    b2_pack = np.ascontiguousarray(np.broadcast_to(
        np.asarray(b_g2, np.float32), (P, D)))
    wc1_pack = np.ascontiguousarray(
        np.asarray(W_c1, np.float32).reshape(KCH, P, 2, P)
        .transpose(1, 0, 2, 3).reshape(P, KCH * 2 * P))
    bc1_pack = np.ascontiguousarray(np.asarray(b_c1, np.float32).reshape(2, P).T)
    wc2_pack = np.ascontiguousarray(np.asarray(W_c2, np.float32).reshape(2, P).T)
    bc2_pack = np.asarray(b_c2, np.float32).reshape(1, 1)

    x_bf = x.astype(F16)
    for c in range(N_CORES):
        idx_flat, slot_flat = per_core[c]

        # x shard padded
        xs = np.zeros((PAD_ROWS, D), F16)
        xs[:ROWS] = x_bf[c * ROWS : (c + 1) * ROWS]

        # one-hot S, packed [128, TILES*k_max*128]: S[p, (t*k_max+k)*128 + d]
        flat_slots = slot_flat.reshape(-1)            # [TILES*k_max*128]
        S = np.zeros((TILES * k_max * P, P), F16)
        valid = flat_slots >= 0
        S[np.nonzero(valid)[0], flat_slots[valid]] = 1
        s_pack = np.ascontiguousarray(
            S.reshape(TILES * k_max, P, P).transpose(1, 0, 2).reshape(P, -1))

        # gather idx table [128, TILES*k_max*8] wrapped per call
        cols = []
        for t in range(TILES):
            off = 0
            for sz in sizes:
                block = idx_flat[t, off * P : (off + sz) * P]
                cols.append(_wrap_idx(block))
                off += sz
        idx16 = np.concatenate(cols, axis=1)          # [16, TILES*k_max*8]
        idx_pack = np.ascontiguousarray(np.tile(idx16, (8, 1)))

        # pooling one-hot [128, TILES*64]
        bvals = batch[c * ROWS : (c + 1) * ROWS]
        Pm = np.zeros((PAD_ROWS, N_GRAPHS), F16)
        Pm[np.arange(ROWS), bvals] = 1
        p_pack = np.ascontiguousarray(
            Pm.reshape(TILES, P, N_GRAPHS).transpose(1, 0, 2).reshape(P, -1))

        in_maps.append({
            "x_sh": xs,
            "idx_all": idx_pack,
            "s_all": s_pack,
            "p_all": p_pack,
            "w1": w1_pack, "w2": w2_pack,
            "b1b": b1_pack, "b2b": b2_pack,
            "wc1": wc1_pack, "bc1": bc1_pack,
            "wc2": wc2_pack, "bc2": bc2_pack,
        })
    return k_max, in_maps


def kernel(**inputs):
    global LAST_EXEC_NS, LAST_RESULTS
    k_max, in_maps = _prep_inputs(**inputs)
    if k_max not in _prog_cache:
        _prog_cache[k_max] = _build_program(k_max)
    nc = _prog_cache[k_max]
    trace = os.environ.get("GNN_TRACE", "0") == "1"
    res = run_bass_kernel_spmd(
        nc, in_maps, core_ids=list(range(N_CORES)), trace=trace,
        tmpdir=os.environ.get("GNN_TRACE_DIR") or None,
    )
    LAST_EXEC_NS = getattr(res, "exec_time_ns", None)
    LAST_RESULTS = res
    return np.asarray(res.results[0]["scores"]).reshape(N_GRAPHS).astype(np.float32)


# revision 8
# speedup vs baseline: 1.1988x; 1.1988x over previous
"""CrossEncoderGNN (2x GIN layer + sum-pool + MLP + sigmoid) on 8 trn2 NeuronCores.

Strategy
--------
Math: GIN layer  h' = (h + A h) @ W + b  ==  (I + A) (h @ W) + b   (A acts on
rows, W on columns, so they commute).  Per layer:
  phase A: y = h @ W computed on each core for its 2500-node shard (dense
           matmul, xbar-transpose DMA provides h^T tiles as lhsT).
  AllGather: y shards (f16) -> full padded table [8*2560, 512] on every core.
  phase B: per dst-tile of 128 nodes, dma_gather the y rows of all incident
           edges (dst-sorted, self-loops included) and segment-sum them with a
           one-hot [128e x 128d] matmul into PSUM; add bias.
Pooling (graph segment-sum) is one more one-hot matmul accumulated over the
core's 20 node tiles; partial pooled [64,512] is AllReduced, and the tiny
classifier MLP + sigmoid runs replicated on every core.

Sharding: nodes (and their incident in-edges) are split 8 ways by contiguous
dst ranges: core c owns nodes [2500c, 2500c+2500), padded to 2560 rows so
every core has 20 uniform tiles of 128.
"""

import sys

for _p in ("/opt/trn_rl_repo", "/root/.axon_site/_ro/trn_rl_repo"):
    if _p not in sys.path:
        sys.path.insert(0, _p)

import os
import numpy as np
import ml_dtypes

import concourse.bass as bass
import concourse.bacc as bacc
import concourse.tile as tile
from concourse import mybir
from concourse.bass_utils import run_bass_kernel_spmd
from concourse.masks import make_identity

F16 = np.float16

N_NODES = 20000
N_EDGES = 320000
D = 512
N_GRAPHS = 64
N_CORES = 8
ROWS = N_NODES // N_CORES          # 2500 real rows per core
P = 128
TILES = (ROWS + P - 1) // P        # 20
PAD_ROWS = TILES * P               # 2560 padded rows per core
FULL_PAD = PAD_ROWS * N_CORES      # 20480
KCH = D // P                       # 4 contraction chunks of 128

LAST_EXEC_NS = None
LAST_RESULTS = None

_prog_cache = {}


def _gather_split(k_max):
    """Split k_max chunks into balanced calls of <=6 chunks each."""
    n_calls = max(1, (k_max + 5) // 6)
    base = k_max // n_calls
    rem = k_max - base * n_calls
    return [base + (1 if i < rem else 0) for i in range(n_calls)]


def _build_program(k_max):
    sizes = _gather_split(k_max)
    kg_max = max(sizes)
    f32 = mybir.dt.float32
    f16 = mybir.dt.float16
    i16 = mybir.dt.int16

    nc = bacc.Bacc("TRN2", debug=False, num_devices=N_CORES, num_swdge_queues=4)

    # ---- I/O ----
    x_sh = nc.dram_tensor("x_sh", [PAD_ROWS, D], f16, kind="ExternalInput")
    idx_all = nc.dram_tensor("idx_all", [P, TILES * k_max * 8], i16, kind="ExternalInput")
    s_all = nc.dram_tensor("s_all", [P, TILES * k_max * P], f16, kind="ExternalInput")
    p_all = nc.dram_tensor("p_all", [P, TILES * N_GRAPHS], f16, kind="ExternalInput")
    w1_in = nc.dram_tensor("w1", [P, KCH * D], f16, kind="ExternalInput")
    w2_in = nc.dram_tensor("w2", [P, KCH * D], f16, kind="ExternalInput")
    b1_in = nc.dram_tensor("b1b", [P, D], f32, kind="ExternalInput")
    b2_in = nc.dram_tensor("b2b", [P, D], f32, kind="ExternalInput")
    wc1_in = nc.dram_tensor("wc1", [P, KCH * 2 * P], f32, kind="ExternalInput")
    bc1_in = nc.dram_tensor("bc1", [P, 2], f32, kind="ExternalInput")
    wc2_in = nc.dram_tensor("wc2", [P, 2], f32, kind="ExternalInput")
    bc2_in = nc.dram_tensor("bc2", [1, 1], f32, kind="ExternalInput")
    scores = nc.dram_tensor("scores", [1, N_GRAPHS], f32, kind="ExternalOutput")

    # ---- internal DRAM ----
    y1_sh = nc.dram_tensor("y1_sh", [PAD_ROWS, D], f16)
    y2_sh = nc.dram_tensor("y2_sh", [PAD_ROWS, D], f16)
    h1_sh = nc.dram_tensor("h1_sh", [PAD_ROWS, D], f16)
    y1_full = nc.dram_tensor("y1_full", [FULL_PAD, D], f16, addr_space="Shared")
    y2_full = nc.dram_tensor("y2_full", [FULL_PAD, D], f16, addr_space="Shared")
    pool_in = nc.dram_tensor("pool_in", [N_GRAPHS, D], f32)
    pool_out = nc.dram_tensor("pool_out", [N_GRAPHS, D], f32, addr_space="Shared")

    rg = [list(range(N_CORES))]

    with tile.TileContext(nc) as tc:
        with (
            tc.tile_pool(name="const", bufs=1) as const,
            tc.tile_pool(name="xT", bufs=1) as xT_pool,
            tc.tile_pool(name="gbuf", bufs=3) as gpool,
            tc.tile_pool(name="stage", bufs=2) as stage_pool,
            tc.tile_pool(name="h2p", bufs=2) as h2_pool,
            tc.tile_pool(name="mlp", bufs=1) as mlp_pool,
            tc.tile_pool(name="psA", bufs=2, space="PSUM") as psA,
            tc.tile_pool(name="psPool", bufs=1, space="PSUM") as psPool,
            tc.tile_pool(name="psMLP", bufs=2, space="PSUM") as psMLP,
        ):
            # ---- resident constants ----
            idx_sb = const.tile([P, TILES * k_max * 8], i16)
            nc.sync.dma_start(out=idx_sb[:], in_=idx_all[:])
            s_flat = const.tile([P, TILES * k_max * P], f16)
            nc.sync.dma_start(out=s_flat[:], in_=s_all[:])
            s_sb = s_flat[:].rearrange("p (c d) -> p c d", d=P)
            p_flat = const.tile([P, TILES * N_GRAPHS], f16)
            nc.sync.dma_start(out=p_flat[:], in_=p_all[:])
            p_sb = p_flat[:].rearrange("p (t g) -> p t g", g=N_GRAPHS)
            w_sb = []
            for w_in in (w1_in, w2_in):
                wt = const.tile([P, KCH * D], f16)
                nc.sync.dma_start(out=wt[:], in_=w_in[:])
                w_sb.append(wt[:].rearrange("p (j d) -> p j d", d=D))
            b_sb = []
            for b_in in (b1_in, b2_in):
                bt = const.tile([P, D], f32)
                nc.sync.dma_start(out=bt[:], in_=b_in[:])
                b_sb.append(bt)
            wc1_sb = const.tile([P, KCH * 2 * P], f32)
            nc.sync.dma_start(out=wc1_sb[:], in_=wc1_in[:])
            wc1_v = wc1_sb[:].rearrange("p (j c m) -> p j c m", c=2, m=P)
            bc1_sb = const.tile([P, 2], f32)
            nc.sync.dma_start(out=bc1_sb[:], in_=bc1_in[:])
            wc2_sb = const.tile([P, 2], f32)
            nc.sync.dma_start(out=wc2_sb[:], in_=wc2_in[:])
            bc2_sb = const.tile([1, 1], f32)
            nc.sync.dma_start(out=bc2_sb[:], in_=bc2_in[:])
            ident = const.tile([P, P], f32)
            make_identity(nc, ident[:])

            def phase_a(h_dram, w_view, y_dram):
                """y = h @ W for this core's 20 row tiles."""
                xT = xT_pool.tile([P, KCH, PAD_ROWS], f16, tag="xT")
                for j in range(KCH):
                    nc.sync.dma_start(
                        out=xT[:, j, :],
                        in_=h_dram[:, j * P : (j + 1) * P],
                        transpose=True,
                    )
                y_view = y_dram.ap().rearrange("(t p) d -> p t d", p=P)
                for grp in range(TILES // 4):
                    st = stage_pool.tile([P, 4, D], f16, tag="stage")
                    for tt in range(4):
                        t = grp * 4 + tt
                        ps = psA.tile([P, D], f32, tag="psA")
                        for j in range(KCH):
                            nc.tensor.matmul(
                                out=ps[:],
                                lhsT=xT[:, j, t * P : (t + 1) * P],
                                rhs=w_view[:, j, :],
                                start=(j == 0),
                                stop=(j == KCH - 1),
                            )
                        nc.vector.tensor_copy(out=st[:, tt, :], in_=ps[:])
                    nc.sync.dma_start(
                        out=y_view[:, grp * 4 : grp * 4 + 4, :], in_=st[:]
                    )

            qn_counter = [0]

            def phase_b_tiles(y_full):
                """Yield (t, agg_psum) for each dst tile."""
                for t in range(TILES):
                    gs = []
                    off = 0
                    for sz in sizes:
                        g = gpool.tile([P, kg_max, D], f16, tag="g")
                        col0 = (t * k_max + off) * 8
                        nc.gpsimd.dma_gather(
                            out_ap=g[:, :sz, :],
                            in_ap=y_full[:],
                            idxs_ap=idx_sb[:, col0 : col0 + sz * 8],
                            num_idxs=sz * P,
                            num_idxs_reg=sz * P,
                            elem_size=D,
                            queue_num=qn_counter[0] % 4,
                        )
                        qn_counter[0] += 1
                        gs.append((g, off, sz))
                        off += sz
                    ps = psA.tile([P, D], f32, tag="psA")
                    ki = 0
                    for g, off, sz in gs:
                        for k in range(sz):
                            nc.tensor.matmul(
                                out=ps[:],
                                lhsT=s_sb[:, t * k_max + off + k, :],
                                rhs=g[:, k, :],
                                start=(ki == 0),
                                stop=(ki == k_max - 1),
                            )
                            ki += 1
                    yield t, ps

            # ---- layer 1 ----
            phase_a(x_sh, w_sb[0], y1_sh)
            nc.gpsimd.collective_compute(
                "AllGather", mybir.AluOpType.bypass, replica_groups=rg,
                ins=[y1_sh[:]], outs=[y1_full[:]],
            )
            h1_view = h1_sh.ap().rearrange("(t p) d -> p t d", p=P)
            st = None
            for t, ps in phase_b_tiles(y1_full):
                if t % 4 == 0:
                    st = stage_pool.tile([P, 4, D], f16, tag="stage")
                nc.vector.tensor_add(out=st[:, t % 4, :], in0=ps[:], in1=b_sb[0][:])
                if t % 4 == 3:
                    g0 = t - 3
                    nc.sync.dma_start(
                        out=h1_view[:, g0 : g0 + 4, :], in_=st[:]
                    )

            # ---- layer 2 ----
            phase_a(h1_sh, w_sb[1], y2_sh)
            nc.gpsimd.collective_compute(
                "AllGather", mybir.AluOpType.bypass, replica_groups=rg,
                ins=[y2_sh[:]], outs=[y2_full[:]],
            )
            pool_ps = psPool.tile([N_GRAPHS, D], f32)
            for t, ps in phase_b_tiles(y2_full):
                h2 = h2_pool.tile([P, D], f16, tag="h2")
                nc.vector.tensor_add(out=h2[:], in0=ps[:], in1=b_sb[1][:])
                nc.tensor.matmul(
                    out=pool_ps[:],
                    lhsT=p_sb[:, t, :],
                    rhs=h2[:],
                    start=(t == 0),
                    stop=(t == TILES - 1),
                    skip_group_check=True,
                )

            # ---- pooled AllReduce ----
            pool_sb = mlp_pool.tile([N_GRAPHS, D], f32)
            nc.vector.tensor_copy(out=pool_sb[:], in_=pool_ps[:])
            nc.sync.dma_start(out=pool_in[:], in_=pool_sb[:])
            nc.gpsimd.collective_compute(
                "AllReduce", mybir.AluOpType.add, replica_groups=rg,
                ins=[pool_in[:]], outs=[pool_out[:]],
            )

            # ---- classifier MLP (replicated, all f32) ----
            pooled = mlp_pool.tile([N_GRAPHS, D], f32)
            nc.sync.dma_start(out=pooled[:], in_=pool_out[:])
            pooledT = mlp_pool.tile([P, KCH, N_GRAPHS], f32)
            for j in range(KCH):
                ps_t = psMLP.tile([P, N_GRAPHS], f32, tag="psT")
                nc.tensor.transpose(
                    out=ps_t[:],
                    in_=pooled[:, j * P : (j + 1) * P],
                    identity=ident[0:N_GRAPHS, 0:N_GRAPHS],
                )
                nc.vector.tensor_copy(out=pooledT[:, j, :], in_=ps_t[:])
            zT = mlp_pool.tile([P, 2, N_GRAPHS], f32)
            for c2 in range(2):
                ps_z = psMLP.tile([P, N_GRAPHS], f32, tag="psT")
                for j in range(KCH):
                    nc.tensor.matmul(
                        out=ps_z[:],
                        lhsT=wc1_v[:, j, c2, :],
                        rhs=pooledT[:, j, :],
                        start=(j == 0),
                        stop=(j == KCH - 1),
                    )
                nc.scalar.activation(
                    out=zT[:, c2, :], in_=ps_z[:],
                    func=mybir.ActivationFunctionType.Relu,
                    bias=bc1_sb[:, c2 : c2 + 1],
                )
            ps_s = psMLP.tile([1, N_GRAPHS], f32, tag="psS")
            for c2 in range(2):
                nc.tensor.matmul(
                    out=ps_s[:],
                    lhsT=wc2_sb[:, c2 : c2 + 1],
                    rhs=zT[:, c2, :],
                    start=(c2 == 0),
                    stop=(c2 == 1),
                )
            score_sb = mlp_pool.tile([1, N_GRAPHS], f32)
            nc.scalar.activation(
                out=score_sb[:], in_=ps_s[:],
                func=mybir.ActivationFunctionType.Sigmoid,
                bias=bc2_sb[0:1, 0:1],
            )
            nc.sync.dma_start(out=scores[:], in_=score_sb[:])

    nc.finalize()
    return nc


def _wrap_idx(block):
    """[n] -> [16, n/16] wrapped: element i at [i%16, i//16]."""
    n = block.shape[0]
    return block.reshape(n // 16, 16).T


def _prep_inputs(joint_x, joint_edge_index, joint_batch,
                 W_g1, b_g1, W_g2, b_g2, W_c1, b_c1, W_c2, b_c2):
    x = np.asarray(joint_x, np.float32)
    ei = np.asarray(joint_edge_index).astype(np.int64)
    batch = np.asarray(joint_batch).astype(np.int64)
    src, dst = ei[0], ei[1]

    # self loops
    all_nodes = np.arange(N_NODES, dtype=np.int64)
    src = np.concatenate([src, all_nodes])
    dst = np.concatenate([dst, all_nodes])

    core_of = dst // ROWS
    local_dst = dst - core_of * ROWS
    psrc = (src // ROWS) * PAD_ROWS + (src % ROWS)   # padded-id space

    # per-(core, tile) edge counts to find k_max
    tile_of = local_dst // P
    key = core_of * TILES + tile_of
    counts = np.bincount(key, minlength=N_CORES * TILES)
    k_max = int((counts.max() + P - 1) // P)

    # Dedupe (src,dst) pairs per tile: the one-hot S entry carries the
    # multiplicity (exact small ints in fp16), shrinking the gather list.
    # key = (core, tile) * |space| + psrc * 128 + slot  uniquely identifies a
    # (gathered row, dst slot) pair.
    tile_key = core_of * TILES + tile_of
    pair_key = (tile_key * FULL_PAD + psrc) * P + (local_dst % P)
    uniq, mult = np.unique(pair_key, return_counts=True)
    u_slot = uniq % P
    u_psrc = (uniq // P) % FULL_PAD
    u_tkey = uniq // (P * FULL_PAD)
    counts = np.bincount(u_tkey, minlength=N_CORES * TILES)
    k_max = int((counts.max() + P - 1) // P)

    per_core = []
    for c in range(N_CORES):
        idx_flat = np.zeros((TILES, k_max * P), np.int16)
        slot_flat = np.full((TILES, k_max * P), -1, np.int64)
        mult_flat = np.zeros((TILES, k_max * P), np.float32)
        m = u_tkey // TILES == c
        ld_s = u_slot[m]
        ps = u_psrc[m]
        mu = mult[m]
        t = u_tkey[m] % TILES
        # np.unique sorted pair_key => already grouped by tile, ascending psrc
        # within each tile (ascending slot within equal psrc).
        cnt = np.bincount(t, minlength=TILES)
        starts = np.concatenate([[0], np.cumsum(cnt)])
        rank = np.arange(len(ps)) - starts[t]
        # Deal the src-sorted list 16 ways so each SDMA engine (descriptor
        # i -> engine i%16) walks ascending HBM addresses.
        n_t = cnt[t]
        sub_len = (n_t + 15) // 16
        pos = (rank % sub_len) * 16 + rank // sub_len
        # pos can exceed n_t-1 when n_t % 16 != 0; it stays < k_max*128.
        idx_flat[t, pos] = ps.astype(np.int16)
        slot_flat[t, pos] = ld_s
        mult_flat[t, pos] = mu
        per_core.append((idx_flat, slot_flat, mult_flat))

    sizes = _gather_split(k_max)

    in_maps = []
    w1_pack = np.ascontiguousarray(
        W_g1.astype(F16).reshape(KCH, P, D).transpose(1, 0, 2).reshape(P, KCH * D))
    w2_pack = np.ascontiguousarray(
        W_g2.astype(F16).reshape(KCH, P, D).transpose(1, 0, 2).reshape(P, KCH * D))
    b1_pack = np.ascontiguousarray(np.broadcast_to(
        np.asarray(b_g1, np.float32), (P, D)))
    b2_pack = np.ascontiguousarray(np.broadcast_to(
        np.asarray(b_g2, np.float32), (P, D)))
    wc1_pack = np.ascontiguousarray(
        np.asarray(W_c1, np.float32).reshape(KCH, P, 2, P)
        .transpose(1, 0, 2, 3).reshape(P, KCH * 2 * P))
    bc1_pack = np.ascontiguousarray(np.asarray(b_c1, np.float32).reshape(2, P).T)
    wc2_pack = np.ascontiguousarray(np.asarray(W_c2, np.float32).reshape(2, P).T)
    bc2_pack = np.asarray(b_c2, np.float32).reshape(1, 1)

    x_bf = x.astype(F16)
    for c in range(N_CORES):
        idx_flat, slot_flat, mult_flat = per_core[c]

        # x shard padded
        xs = np.zeros((PAD_ROWS, D), F16)
        xs[:ROWS] = x_bf[c * ROWS : (c + 1) * ROWS]

        # one-hot S, packed [128, TILES*k_max*128]: S[p, (t*k_max+k)*128 + d]
        flat_slots = slot_flat.reshape(-1)            # [TILES*k_max*128]
        flat_mult = mult_flat.reshape(-1)
        S = np.zeros((TILES * k_max * P, P), F16)
        valid = flat_slots >= 0
        S[np.nonzero(valid)[0], flat_slots[valid]] = flat_mult[valid]
        s_pack = np.ascontiguousarray(
            S.reshape(TILES * k_max, P, P).transpose(1, 0, 2).reshape(P, -1))

        # gather idx table [128, TILES*k_max*8] wrapped per call
        cols = []
        for t in range(TILES):
            off = 0
            for sz in sizes:
                block = idx_flat[t, off * P : (off + sz) * P]
                cols.append(_wrap_idx(block))
                off += sz
        idx16 = np.concatenate(cols, axis=1)          # [16, TILES*k_max*8]
        idx_pack = np.ascontiguousarray(np.tile(idx16, (8, 1)))

        # pooling one-hot [128, TILES*64]
        bvals = batch[c * ROWS : (c + 1) * ROWS]
        Pm = np.zeros((PAD_ROWS, N_GRAPHS), F16)
        Pm[np.arange(ROWS), bvals] = 1
        p_pack = np.ascontiguousarray(
            Pm.reshape(TILES, P, N_GRAPHS).transpose(1, 0, 2).reshape(P, -1))

        in_maps.append({
            "x_sh": xs,
            "idx_all": idx_pack,
            "s_all": s_pack,
            "p_all": p_pack,
            "w1": w1_pack, "w2": w2_pack,
            "b1b": b1_pack, "b2b": b2_pack,
            "wc1": wc1_pack, "bc1": bc1_pack,
            "wc2": wc2_pack, "bc2": bc2_pack,
        })
    return k_max, in_maps


def kernel(**inputs):
    global LAST_EXEC_NS, LAST_RESULTS
    k_max, in_maps = _prep_inputs(**inputs)
    if k_max not in _prog_cache:
        _prog_cache[k_max] = _build_program(k_max)
    nc = _prog_cache[k_max]
    trace = os.environ.get("GNN_TRACE", "0") == "1"
    res = run_bass_kernel_spmd(
        nc, in_maps, core_ids=list(range(N_CORES)), trace=trace,
        tmpdir=os.environ.get("GNN_TRACE_DIR") or None,
    )
    LAST_EXEC_NS = getattr(res, "exec_time_ns", None)
    LAST_RESULTS = res
    return np.asarray(res.results[0]["scores"]).reshape(N_GRAPHS).astype(np.float32)


# revision 15
# speedup vs baseline: 1.3890x; 1.1587x over previous
"""CrossEncoderGNN (2x GIN layer + sum-pool + MLP + sigmoid) on 8 trn2 NeuronCores.

Strategy
--------
Math: GIN layer  h' = (h + A h) @ W + b  ==  (I + A) (h @ W) + b   (A acts on
rows, W on columns, so they commute).  Per layer:
  phase A: y = h @ W computed on each core for its 2500-node shard (dense
           matmul, xbar-transpose DMA provides h^T tiles as lhsT).
  AllGather: y shards (f16) -> full padded table [8*2560, 512] on every core.
  phase B: per dst-tile of 128 nodes, dma_gather the y rows of all incident
           edges (dst-sorted, self-loops included) and segment-sum them with a
           one-hot [128e x 128d] matmul into PSUM; add bias.
Pooling (graph segment-sum) is one more one-hot matmul accumulated over the
core's 20 node tiles; partial pooled [64,512] is AllReduced, and the tiny
classifier MLP + sigmoid runs replicated on every core.

Sharding: nodes (and their incident in-edges) are split 8 ways by contiguous
dst ranges: core c owns nodes [2500c, 2500c+2500), padded to 2560 rows so
every core has 20 uniform tiles of 128.
"""

import sys

for _p in ("/opt/trn_rl_repo", "/root/.axon_site/_ro/trn_rl_repo"):
    if _p not in sys.path:
        sys.path.insert(0, _p)

import os
import numpy as np
import ml_dtypes

import concourse.bass as bass
import concourse.bacc as bacc
import concourse.tile as tile
from concourse import mybir
from concourse.bass_utils import run_bass_kernel_spmd
from concourse.masks import make_identity

F16 = np.float16

N_NODES = 20000
N_EDGES = 320000
D = 512
N_GRAPHS = 64
N_CORES = 8
ROWS = N_NODES // N_CORES          # 2500 real rows per core
P = 128
TILES = (ROWS + P - 1) // P        # 20
PAD_ROWS = TILES * P               # 2560 padded rows per core
FULL_PAD = PAD_ROWS * N_CORES      # 20480
KCH = D // P                       # 4 contraction chunks of 128

LAST_EXEC_NS = None
LAST_RESULTS = None

_prog_cache = {}


def _gather_split(k_max):
    """Split k_max chunks into balanced calls of <=8 chunks each."""
    n_calls = max(1, (k_max + 7) // 8)
    base = k_max // n_calls
    rem = k_max - base * n_calls
    return [base + (1 if i < rem else 0) for i in range(n_calls)]


def _build_program(k_max):
    sizes = _gather_split(k_max)
    kg_max = max(sizes)
    f32 = mybir.dt.float32
    f16 = mybir.dt.float16
    i16 = mybir.dt.int16

    nc = bacc.Bacc("TRN2", debug=False, num_devices=N_CORES, num_swdge_queues=4)

    # ---- I/O ----
    x_sh = nc.dram_tensor("x_sh", [PAD_ROWS, D], f16, kind="ExternalInput")
    idx_all = nc.dram_tensor("idx_all", [P, TILES * k_max * 8], i16, kind="ExternalInput")
    s_all = nc.dram_tensor("s_all", [P, TILES * k_max * P], f16, kind="ExternalInput")
    p_all = nc.dram_tensor("p_all", [P, TILES * N_GRAPHS], f16, kind="ExternalInput")
    w1_in = nc.dram_tensor("w1", [P, KCH * D], f16, kind="ExternalInput")
    w2_in = nc.dram_tensor("w2", [P, KCH * D], f16, kind="ExternalInput")
    b1_in = nc.dram_tensor("b1b", [P, D], f32, kind="ExternalInput")
    b2_in = nc.dram_tensor("b2b", [P, D], f32, kind="ExternalInput")
    wc1_in = nc.dram_tensor("wc1", [P, KCH * 2 * P], f32, kind="ExternalInput")
    bc1_in = nc.dram_tensor("bc1", [P, 2], f32, kind="ExternalInput")
    wc2_in = nc.dram_tensor("wc2", [P, 2], f32, kind="ExternalInput")
    bc2_in = nc.dram_tensor("bc2", [1, 1], f32, kind="ExternalInput")
    scores = nc.dram_tensor("scores", [1, N_GRAPHS], f32, kind="ExternalOutput")

    # ---- internal DRAM ----
    y1_sh = nc.dram_tensor("y1_sh", [PAD_ROWS, D], f16)
    y2_sh = nc.dram_tensor("y2_sh", [PAD_ROWS, D], f16)
    h1_sh = nc.dram_tensor("h1_sh", [PAD_ROWS, D], f16)
    y1_full = nc.dram_tensor("y1_full", [FULL_PAD, D], f16, addr_space="Shared")
    y2_full = nc.dram_tensor("y2_full", [FULL_PAD, D], f16, addr_space="Shared")
    pool_in = nc.dram_tensor("pool_in", [N_GRAPHS, D], f32)
    pool_out = nc.dram_tensor("pool_out", [N_GRAPHS, D], f32, addr_space="Shared")

    rg = [list(range(N_CORES))]

    with tile.TileContext(nc) as tc:
        with (
            tc.tile_pool(name="const", bufs=1) as const,
            tc.tile_pool(name="xT", bufs=1) as xT_pool,
            tc.tile_pool(name="gbuf", bufs=3) as gpool,
            tc.tile_pool(name="stage", bufs=2) as stage_pool,
            tc.tile_pool(name="h2p", bufs=2) as h2_pool,
            tc.tile_pool(name="mlp", bufs=1) as mlp_pool,
            tc.tile_pool(name="psA", bufs=2, space="PSUM") as psA,
            tc.tile_pool(name="psPool", bufs=1, space="PSUM") as psPool,
            tc.tile_pool(name="psMLP", bufs=2, space="PSUM") as psMLP,
        ):
            # ---- resident constants ----
            # Bulk loads go through the ACT HWDGE ring (nc.scalar) so they
            # don't serialize with phase A's xbar transposes on the SP ring.
            idx_sb = const.tile([P, TILES * k_max * 8], i16)
            nc.scalar.dma_start(out=idx_sb[:], in_=idx_all[:])
            s_flat = const.tile([P, TILES * k_max * P], f16)
            nc.scalar.dma_start(out=s_flat[:], in_=s_all[:])
            s_sb = s_flat[:].rearrange("p (c d) -> p c d", d=P)
            p_flat = const.tile([P, TILES * N_GRAPHS], f16)
            nc.scalar.dma_start(out=p_flat[:], in_=p_all[:])
            p_sb = p_flat[:].rearrange("p (t g) -> p t g", g=N_GRAPHS)
            w_sb = []
            for w_in in (w1_in, w2_in):
                wt = const.tile([P, KCH * D], f16)
                nc.sync.dma_start(out=wt[:], in_=w_in[:])
                w_sb.append(wt[:].rearrange("p (j d) -> p j d", d=D))
            b_sb = []
            for b_in in (b1_in, b2_in):
                bt = const.tile([P, D], f32)
                nc.scalar.dma_start(out=bt[:], in_=b_in[:])
                b_sb.append(bt)
            wc1_sb = const.tile([P, KCH * 2 * P], f32)
            nc.scalar.dma_start(out=wc1_sb[:], in_=wc1_in[:])
            wc1_v = wc1_sb[:].rearrange("p (j c m) -> p j c m", c=2, m=P)
            bc1_sb = const.tile([P, 2], f32)
            nc.scalar.dma_start(out=bc1_sb[:], in_=bc1_in[:])
            wc2_sb = const.tile([P, 2], f32)
            nc.scalar.dma_start(out=wc2_sb[:], in_=wc2_in[:])
            bc2_sb = const.tile([1, 1], f32)
            nc.scalar.dma_start(out=bc2_sb[:], in_=bc2_in[:])
            ident = const.tile([P, P], f32)
            make_identity(nc, ident[:])

            def phase_a(h_dram, w_view, y_dram):
                """y = h @ W for this core's 20 row tiles."""
                xT = xT_pool.tile([P, KCH, PAD_ROWS], f16, tag="xT")
                for j in range(KCH):
                    nc.sync.dma_start(
                        out=xT[:, j, :],
                        in_=h_dram[:, j * P : (j + 1) * P],
                        transpose=True,
                    )
                y_view = y_dram.ap().rearrange("(t p) d -> p t d", p=P)
                for grp in range(TILES // 4):
                    st = stage_pool.tile([P, 4, D], f16, tag="stage")
                    for tt in range(4):
                        t = grp * 4 + tt
                        ps = psA.tile([P, D], f32, tag="psA")
                        for j in range(KCH):
                            nc.tensor.matmul(
                                out=ps[:],
                                lhsT=xT[:, j, t * P : (t + 1) * P],
                                rhs=w_view[:, j, :],
                                start=(j == 0),
                                stop=(j == KCH - 1),
                            )
                        nc.vector.tensor_copy(out=st[:, tt, :], in_=ps[:])
                    nc.sync.dma_start(
                        out=y_view[:, grp * 4 : grp * 4 + 4, :], in_=st[:]
                    )

            qn_counter = [0]

            def phase_b_tiles(y_full, y_sh):
                """Yield (t, agg_psum, y_own) for each dst tile.

                y_own is the tile's own 128 rows of y, loaded contiguously
                from the local shard (the GIN self term), to be DVE-added by
                the consumer along with the bias.
                """
                for t in range(TILES):
                    y_own = h2_pool.tile([P, D], f16, tag="yown")
                    nc.sync.dma_start(
                        out=y_own[:], in_=y_sh[t * P : (t + 1) * P, :]
                    )
                    gs = []
                    off = 0
                    for sz in sizes:
                        g = gpool.tile([P, kg_max, D], f16, tag="g")
                        col0 = (t * k_max + off) * 8
                        nc.gpsimd.dma_gather(
                            out_ap=g[:, :sz, :],
                            in_ap=y_full[:],
                            idxs_ap=idx_sb[:, col0 : col0 + sz * 8],
                            num_idxs=sz * P,
                            num_idxs_reg=sz * P,
                            elem_size=D,
                            queue_num=qn_counter[0] % 4,
                        )
                        qn_counter[0] += 1
                        gs.append((g, off, sz))
                        off += sz
                    ps = psA.tile([P, D], f32, tag="psA")
                    ki = 0
                    for g, off, sz in gs:
                        for k in range(sz):
                            nc.tensor.matmul(
                                out=ps[:],
                                lhsT=s_sb[:, t * k_max + off + k, :],
                                rhs=g[:, k, :],
                                start=(ki == 0),
                                stop=(ki == k_max - 1),
                            )
                            ki += 1
                    yield t, ps, y_own

            # ---- layer 1 ----
            phase_a(x_sh, w_sb[0], y1_sh)
            nc.gpsimd.collective_compute(
                "AllGather", mybir.AluOpType.bypass, replica_groups=rg,
                ins=[y1_sh[:]], outs=[y1_full[:]],
            )
            h1_view = h1_sh.ap().rearrange("(t p) d -> p t d", p=P)
            st = None
            for t, ps, y_own in phase_b_tiles(y1_full, y1_sh):
                if t % 4 == 0:
                    st = stage_pool.tile([P, 4, D], f16, tag="stage")
                nc.vector.tensor_add(out=st[:, t % 4, :], in0=ps[:], in1=b_sb[0][:])
                nc.vector.tensor_add(
                    out=st[:, t % 4, :], in0=st[:, t % 4, :], in1=y_own[:]
                )
                if t % 4 == 3:
                    g0 = t - 3
                    nc.sync.dma_start(
                        out=h1_view[:, g0 : g0 + 4, :], in_=st[:]
                    )

            # ---- layer 2 ----
            phase_a(h1_sh, w_sb[1], y2_sh)
            nc.gpsimd.collective_compute(
                "AllGather", mybir.AluOpType.bypass, replica_groups=rg,
                ins=[y2_sh[:]], outs=[y2_full[:]],
            )
            pool_ps = psPool.tile([N_GRAPHS, D], f32)
            for t, ps, y_own in phase_b_tiles(y2_full, y2_sh):
                h2 = h2_pool.tile([P, D], f16, tag="h2")
                nc.vector.tensor_add(out=h2[:], in0=ps[:], in1=b_sb[1][:])
                nc.vector.tensor_add(out=h2[:], in0=h2[:], in1=y_own[:])
                nc.tensor.matmul(
                    out=pool_ps[:],
                    lhsT=p_sb[:, t, :],
                    rhs=h2[:],
                    start=(t == 0),
                    stop=(t == TILES - 1),
                    skip_group_check=True,
                )

            # ---- pooled AllReduce ----
            pool_sb = mlp_pool.tile([N_GRAPHS, D], f32)
            nc.vector.tensor_copy(out=pool_sb[:], in_=pool_ps[:])
            nc.sync.dma_start(out=pool_in[:], in_=pool_sb[:])
            nc.gpsimd.collective_compute(
                "AllReduce", mybir.AluOpType.add, replica_groups=rg,
                ins=[pool_in[:]], outs=[pool_out[:]],
            )

            # ---- classifier MLP (replicated, all f32) ----
            pooled = mlp_pool.tile([N_GRAPHS, D], f32)
            nc.sync.dma_start(out=pooled[:], in_=pool_out[:])
            pooledT = mlp_pool.tile([P, KCH, N_GRAPHS], f32)
            for j in range(KCH):
                ps_t = psMLP.tile([P, N_GRAPHS], f32, tag="psT")
                nc.tensor.transpose(
                    out=ps_t[:],
                    in_=pooled[:, j * P : (j + 1) * P],
                    identity=ident[0:N_GRAPHS, 0:N_GRAPHS],
                )
                nc.vector.tensor_copy(out=pooledT[:, j, :], in_=ps_t[:])
            zT = mlp_pool.tile([P, 2, N_GRAPHS], f32)
            for c2 in range(2):
                ps_z = psMLP.tile([P, N_GRAPHS], f32, tag="psT")
                for j in range(KCH):
                    nc.tensor.matmul(
                        out=ps_z[:],
                        lhsT=wc1_v[:, j, c2, :],
                        rhs=pooledT[:, j, :],
                        start=(j == 0),
                        stop=(j == KCH - 1),
                    )
                nc.scalar.activation(
                    out=zT[:, c2, :], in_=ps_z[:],
                    func=mybir.ActivationFunctionType.Relu,
                    bias=bc1_sb[:, c2 : c2 + 1],
                )
            ps_s = psMLP.tile([1, N_GRAPHS], f32, tag="psS")
            for c2 in range(2):
                nc.tensor.matmul(
                    out=ps_s[:],
                    lhsT=wc2_sb[:, c2 : c2 + 1],
                    rhs=zT[:, c2, :],
                    start=(c2 == 0),
                    stop=(c2 == 1),
                )
            score_sb = mlp_pool.tile([1, N_GRAPHS], f32)
            nc.scalar.activation(
                out=score_sb[:], in_=ps_s[:],
                func=mybir.ActivationFunctionType.Sigmoid,
                bias=bc2_sb[0:1, 0:1],
            )
            nc.sync.dma_start(out=scores[:], in_=score_sb[:])

    nc.finalize()
    return nc


def _wrap_idx(block):
    """[n] -> [16, n/16] wrapped: element i at [i%16, i//16]."""
    n = block.shape[0]
    return block.reshape(n // 16, 16).T


def _prep_inputs(joint_x, joint_edge_index, joint_batch,
                 W_g1, b_g1, W_g2, b_g2, W_c1, b_c1, W_c2, b_c2):
    import heapq

    x = np.asarray(joint_x, np.float32)
    ei = np.asarray(joint_edge_index).astype(np.int64)
    batch = np.asarray(joint_batch).astype(np.int64)
    src, dst = ei[0], ei[1]

    # Unique (src,dst) pairs; multiplicity rides in the S matrix (exact small
    # ints in fp16). Self term (I+A diagonal) is handled separately on-device
    # via a contiguous load of the tile's own y rows, so no self-loop edges.
    pk = src * N_NODES + dst
    upair, mult = np.unique(pk, return_counts=True)
    u_src = upair // N_NODES
    u_dst = upair % N_NODES

    # Rebalance: assign dst nodes to the 160 (core,tile) bins, greedily
    # equalizing per-bin in-edge counts, so every tile needs the same (and
    # minimal) number of 128-edge chunks. The node->position permutation is
    # free to choose: pooling only needs each node's graph id.
    indeg = np.bincount(u_dst, minlength=N_NODES)
    n_bins = N_CORES * TILES
    order = np.argsort(-indeg, kind="stable")
    heap = [(0, b) for b in range(n_bins)]
    heapq.heapify(heap)
    cap = np.full(n_bins, P, np.int64)
    node_bin = np.empty(N_NODES, np.int64)
    node_slot = np.empty(N_NODES, np.int64)
    for n in order:
        while True:
            load, b = heapq.heappop(heap)
            if cap[b] > 0:
                break
        node_bin[n] = b
        node_slot[n] = P - cap[b]
        cap[b] -= 1
        heapq.heappush(heap, (load + int(indeg[n]), b))
    pos = (node_bin // TILES) * PAD_ROWS + (node_bin % TILES) * P + node_slot

    # Gather rows: one per unique (dst-bin, src) — a single gathered y row
    # feeds every dst slot of that tile that has an edge from src.
    bin_of_pair = node_bin[u_dst]
    rk = bin_of_pair * FULL_PAD + pos[u_src]
    urow, row_inv = np.unique(rk, return_inverse=True)
    row_bin = urow // FULL_PAD
    row_psrc = urow % FULL_PAD
    rows_per_bin = np.bincount(row_bin, minlength=n_bins)
    k_max = int((rows_per_bin.max() + P - 1) // P)
    sizes = _gather_split(k_max)

    # Rank within bin (urow sorted => grouped by bin, ascending src pos),
    # then deal 16 ways so each SDMA engine (descriptor i -> engine i%16)
    # walks ascending HBM addresses.
    bin_starts = np.concatenate([[0], np.cumsum(rows_per_bin)])
    row_rank = np.arange(len(urow)) - bin_starts[row_bin]
    n_b = rows_per_bin[row_bin]
    sub_len = (n_b + 15) // 16
    row_pos = (row_rank % sub_len) * 16 + row_rank // sub_len

    per_core = []
    pair_bin = bin_of_pair
    pair_rowpos = row_pos[row_inv]
    pair_slot = node_slot[u_dst]
    for c in range(N_CORES):
        idx_flat = np.zeros((TILES, k_max * P), np.int16)
        m = row_bin // TILES == c
        t = row_bin[m] % TILES
        idx_flat[t, row_pos[m]] = row_psrc[m].astype(np.int16)
        # S: [TILES*k_max*128 rows, 128 slots]
        S = np.zeros((TILES * k_max * P, P), F16)
        pm = pair_bin // TILES == c
        pt = pair_bin[pm] % TILES
        srow = pt * (k_max * P) + pair_rowpos[pm]
        S[srow, pair_slot[pm]] = mult[pm]
        per_core.append((idx_flat, S))

    # node at each padded position (for x shard + pooling construction)
    node_at = np.full(N_CORES * PAD_ROWS, -1, np.int64)
    node_at[pos] = np.arange(N_NODES)

    in_maps = []
    w1_pack = np.ascontiguousarray(
        W_g1.astype(F16).reshape(KCH, P, D).transpose(1, 0, 2).reshape(P, KCH * D))
    w2_pack = np.ascontiguousarray(
        W_g2.astype(F16).reshape(KCH, P, D).transpose(1, 0, 2).reshape(P, KCH * D))
    b1_pack = np.ascontiguousarray(np.broadcast_to(
        np.asarray(b_g1, np.float32), (P, D)))
    b2_pack = np.ascontiguousarray(np.broadcast_to(
        np.asarray(b_g2, np.float32), (P, D)))
    wc1_pack = np.ascontiguousarray(
        np.asarray(W_c1, np.float32).reshape(KCH, P, 2, P)
        .transpose(1, 0, 2, 3).reshape(P, KCH * 2 * P))
    bc1_pack = np.ascontiguousarray(np.asarray(b_c1, np.float32).reshape(2, P).T)
    wc2_pack = np.ascontiguousarray(np.asarray(W_c2, np.float32).reshape(2, P).T)
    bc2_pack = np.asarray(b_c2, np.float32).reshape(1, 1)

    x_bf = x.astype(F16)
    for c in range(N_CORES):
        idx_flat, S = per_core[c]

        # x shard in permuted position space
        nodes_c = node_at[c * PAD_ROWS : (c + 1) * PAD_ROWS]
        real = nodes_c >= 0
        xs = np.zeros((PAD_ROWS, D), F16)
        xs[real] = x_bf[nodes_c[real]]

        s_pack = np.ascontiguousarray(
            S.reshape(TILES * k_max, P, P).transpose(1, 0, 2).reshape(P, -1))

        # gather idx table [128, TILES*k_max*8] wrapped per call
        cols = []
        for t in range(TILES):
            off = 0
            for sz in sizes:
                block = idx_flat[t, off * P : (off + sz) * P]
                cols.append(_wrap_idx(block))
                off += sz
        idx16 = np.concatenate(cols, axis=1)          # [16, TILES*k_max*8]
        idx_pack = np.ascontiguousarray(np.tile(idx16, (8, 1)))

        # pooling one-hot [128, TILES*64]
        Pm = np.zeros((PAD_ROWS, N_GRAPHS), F16)
        Pm[real, batch[nodes_c[real]]] = 1
        p_pack = np.ascontiguousarray(
            Pm.reshape(TILES, P, N_GRAPHS).transpose(1, 0, 2).reshape(P, -1))

        in_maps.append({
            "x_sh": xs,
            "idx_all": idx_pack,
            "s_all": s_pack,
            "p_all": p_pack,
            "w1": w1_pack, "w2": w2_pack,
            "b1b": b1_pack, "b2b": b2_pack,
            "wc1": wc1_pack, "bc1": bc1_pack,
            "wc2": wc2_pack, "bc2": bc2_pack,
        })
    return k_max, in_maps


def kernel(**inputs):
    global LAST_EXEC_NS, LAST_RESULTS
    k_max, in_maps = _prep_inputs(**inputs)
    if k_max not in _prog_cache:
        _prog_cache[k_max] = _build_program(k_max)
    nc = _prog_cache[k_max]
    trace = os.environ.get("GNN_TRACE", "0") == "1"
    res = run_bass_kernel_spmd(
        nc, in_maps, core_ids=list(range(N_CORES)), trace=trace,
        tmpdir=os.environ.get("GNN_TRACE_DIR") or None,
    )
    LAST_EXEC_NS = getattr(res, "exec_time_ns", None)
    LAST_RESULTS = res
    return np.asarray(res.results[0]["scores"]).reshape(N_GRAPHS).astype(np.float32)


# revision 17
# speedup vs baseline: 1.4319x; 1.0309x over previous
"""CrossEncoderGNN (2x GIN layer + sum-pool + MLP + sigmoid) on 8 trn2 NeuronCores.

Strategy
--------
Math: GIN layer  h' = (h + A h) @ W + b  ==  (I + A) (h @ W) + b   (A acts on
rows, W on columns, so they commute).  Per layer:
  phase A: y = h @ W computed on each core for its 2500-node shard (dense
           matmul, xbar-transpose DMA provides h^T tiles as lhsT).
  AllGather: y shards (f16) -> full padded table [8*2560, 512] on every core.
  phase B: per dst-tile of 128 nodes, dma_gather the y rows of all incident
           edges (dst-sorted, self-loops included) and segment-sum them with a
           one-hot [128e x 128d] matmul into PSUM; add bias.
Pooling (graph segment-sum) is one more one-hot matmul accumulated over the
core's 20 node tiles; partial pooled [64,512] is AllReduced, and the tiny
classifier MLP + sigmoid runs replicated on every core.

Sharding: nodes (and their incident in-edges) are split 8 ways by contiguous
dst ranges: core c owns nodes [2500c, 2500c+2500), padded to 2560 rows so
every core has 20 uniform tiles of 128.
"""

import sys

for _p in ("/opt/trn_rl_repo", "/root/.axon_site/_ro/trn_rl_repo"):
    if _p not in sys.path:
        sys.path.insert(0, _p)

import os
import numpy as np
import ml_dtypes

import concourse.bass as bass
import concourse.bacc as bacc
import concourse.tile as tile
from concourse import mybir
from concourse.bass_utils import run_bass_kernel_spmd
from concourse.masks import make_identity

F16 = np.float16

N_NODES = 20000
N_EDGES = 320000
D = 512
N_GRAPHS = 64
N_CORES = 8
ROWS = N_NODES // N_CORES          # 2500 real rows per core
P = 128
TILES = (ROWS + P - 1) // P        # 20
PAD_ROWS = TILES * P               # 2560 padded rows per core
FULL_PAD = PAD_ROWS * N_CORES      # 20480
KCH = D // P                       # 4 contraction chunks of 128

LAST_EXEC_NS = None
LAST_RESULTS = None

_prog_cache = {}


def _gather_split(k_max):
    """Split k_max chunks into balanced calls of <=8 chunks each."""
    n_calls = max(1, (k_max + 7) // 8)
    base = k_max // n_calls
    rem = k_max - base * n_calls
    return [base + (1 if i < rem else 0) for i in range(n_calls)]


def _build_program(k_max):
    sizes = _gather_split(k_max)
    kg_max = max(sizes)
    f32 = mybir.dt.float32
    f16 = mybir.dt.float16
    i16 = mybir.dt.int16

    nc = bacc.Bacc("TRN2", debug=False, num_devices=N_CORES, num_swdge_queues=4)

    # ---- I/O ----
    x_sh = nc.dram_tensor("x_sh", [PAD_ROWS, D], f16, kind="ExternalInput")
    idx_all = nc.dram_tensor("idx_all", [P, TILES * k_max * 8], i16, kind="ExternalInput")
    s_all = nc.dram_tensor("s_all", [P, TILES * k_max * P], f16, kind="ExternalInput")
    p_all = nc.dram_tensor("p_all", [P, TILES * N_GRAPHS], f16, kind="ExternalInput")
    w1_in = nc.dram_tensor("w1", [P, KCH * D], f16, kind="ExternalInput")
    w2_in = nc.dram_tensor("w2", [P, KCH * D], f16, kind="ExternalInput")
    b1_in = nc.dram_tensor("b1b", [P, D], f32, kind="ExternalInput")
    b2_in = nc.dram_tensor("b2b", [P, D], f32, kind="ExternalInput")
    wc1_in = nc.dram_tensor("wc1", [P, KCH * 2 * P], f32, kind="ExternalInput")
    bc1_in = nc.dram_tensor("bc1", [P, 2], f32, kind="ExternalInput")
    wc2_in = nc.dram_tensor("wc2", [P, 2], f32, kind="ExternalInput")
    bc2_in = nc.dram_tensor("bc2", [1, 1], f32, kind="ExternalInput")
    scores = nc.dram_tensor("scores", [1, N_GRAPHS], f32, kind="ExternalOutput")

    # ---- internal DRAM ----
    y1_sh = nc.dram_tensor("y1_sh", [PAD_ROWS, D], f16)
    y2_sh = nc.dram_tensor("y2_sh", [PAD_ROWS, D], f16)
    h1_sh = nc.dram_tensor("h1_sh", [PAD_ROWS, D], f16)
    y1_full = nc.dram_tensor("y1_full", [FULL_PAD, D], f16, addr_space="Shared")
    y2_full = nc.dram_tensor("y2_full", [FULL_PAD, D], f16, addr_space="Shared")
    pool_in = nc.dram_tensor("pool_in", [N_GRAPHS, D], f32)
    pool_out = nc.dram_tensor("pool_out", [N_GRAPHS, D], f32, addr_space="Shared")

    rg = [list(range(N_CORES))]

    with tile.TileContext(nc) as tc:
        with (
            tc.tile_pool(name="const", bufs=1) as const,
            tc.tile_pool(name="xT", bufs=1) as xT_pool,
            tc.tile_pool(name="gbuf", bufs=6) as gpool,
            tc.tile_pool(name="stage", bufs=3) as stage_pool,
            tc.tile_pool(name="h2p", bufs=4) as h2_pool,
            tc.tile_pool(name="mlp", bufs=1) as mlp_pool,
            tc.tile_pool(name="psA", bufs=4, space="PSUM") as psA,
            tc.tile_pool(name="psPool", bufs=1, space="PSUM") as psPool,
            tc.tile_pool(name="psMLP", bufs=1, space="PSUM") as psMLP,
        ):
            # ---- resident constants ----
            # Bulk loads go through the ACT HWDGE ring (nc.scalar) so they
            # don't serialize with phase A's xbar transposes on the SP ring.
            idx_sb = const.tile([P, TILES * k_max * 8], i16)
            nc.scalar.dma_start(out=idx_sb[:], in_=idx_all[:])
            s_flat = const.tile([P, TILES * k_max * P], f16)
            nc.scalar.dma_start(out=s_flat[:], in_=s_all[:])
            s_sb = s_flat[:].rearrange("p (c d) -> p c d", d=P)
            p_flat = const.tile([P, TILES * N_GRAPHS], f16)
            nc.scalar.dma_start(out=p_flat[:], in_=p_all[:])
            p_sb = p_flat[:].rearrange("p (t g) -> p t g", g=N_GRAPHS)
            w_sb = []
            for w_in in (w1_in, w2_in):
                wt = const.tile([P, KCH * D], f16)
                nc.sync.dma_start(out=wt[:], in_=w_in[:])
                w_sb.append(wt[:].rearrange("p (j d) -> p j d", d=D))
            b_sb = []
            for b_in in (b1_in, b2_in):
                bt = const.tile([P, D], f32)
                nc.scalar.dma_start(out=bt[:], in_=b_in[:])
                b_sb.append(bt)
            wc1_sb = const.tile([P, KCH * 2 * P], f32)
            nc.scalar.dma_start(out=wc1_sb[:], in_=wc1_in[:])
            wc1_v = wc1_sb[:].rearrange("p (j c m) -> p j c m", c=2, m=P)
            bc1_sb = const.tile([P, 2], f32)
            nc.scalar.dma_start(out=bc1_sb[:], in_=bc1_in[:])
            wc2_sb = const.tile([P, 2], f32)
            nc.scalar.dma_start(out=wc2_sb[:], in_=wc2_in[:])
            bc2_sb = const.tile([1, 1], f32)
            nc.scalar.dma_start(out=bc2_sb[:], in_=bc2_in[:])
            ident = const.tile([P, P], f32)
            make_identity(nc, ident[:])

            def phase_a(h_dram, w_view, y_dram):
                """y = h @ W for this core's 20 row tiles."""
                xT = xT_pool.tile([P, KCH, PAD_ROWS], f16, tag="xT")
                for j in range(KCH):
                    nc.sync.dma_start(
                        out=xT[:, j, :],
                        in_=h_dram[:, j * P : (j + 1) * P],
                        transpose=True,
                    )
                y_view = y_dram.ap().rearrange("(t p) d -> p t d", p=P)
                for grp in range(TILES // 4):
                    st = stage_pool.tile([P, 4, D], f16, tag="stage")
                    for tt in range(4):
                        t = grp * 4 + tt
                        ps = psA.tile([P, D], f32, tag="psA")
                        for j in range(KCH):
                            nc.tensor.matmul(
                                out=ps[:],
                                lhsT=xT[:, j, t * P : (t + 1) * P],
                                rhs=w_view[:, j, :],
                                start=(j == 0),
                                stop=(j == KCH - 1),
                            )
                        nc.vector.tensor_copy(out=st[:, tt, :], in_=ps[:])
                    nc.sync.dma_start(
                        out=y_view[:, grp * 4 : grp * 4 + 4, :], in_=st[:]
                    )

            qn_counter = [0]

            def phase_b_tiles(y_full, y_sh):
                """Yield (t, agg_psum, y_own) for each dst tile.

                y_own is the tile's own 128 rows of y, loaded contiguously
                from the local shard (the GIN self term), to be DVE-added by
                the consumer along with the bias.
                """
                for t in range(TILES):
                    y_own = h2_pool.tile([P, D], f16, tag="yown")
                    nc.sync.dma_start(
                        out=y_own[:], in_=y_sh[t * P : (t + 1) * P, :]
                    )
                    gs = []
                    off = 0
                    for sz in sizes:
                        g = gpool.tile([P, kg_max, D], f16, tag="g")
                        col0 = (t * k_max + off) * 8
                        nc.gpsimd.dma_gather(
                            out_ap=g[:, :sz, :],
                            in_ap=y_full[:],
                            idxs_ap=idx_sb[:, col0 : col0 + sz * 8],
                            num_idxs=sz * P,
                            num_idxs_reg=sz * P,
                            elem_size=D,
                            queue_num=qn_counter[0] % 4,
                        )
                        qn_counter[0] += 1
                        gs.append((g, off, sz))
                        off += sz
                    ps = psA.tile([P, D], f32, tag="psA")
                    ki = 0
                    for g, off, sz in gs:
                        for k in range(sz):
                            nc.tensor.matmul(
                                out=ps[:],
                                lhsT=s_sb[:, t * k_max + off + k, :],
                                rhs=g[:, k, :],
                                start=(ki == 0),
                                stop=(ki == k_max - 1),
                            )
                            ki += 1
                    yield t, ps, y_own

            # ---- layer 1 ----
            phase_a(x_sh, w_sb[0], y1_sh)
            nc.gpsimd.collective_compute(
                "AllGather", mybir.AluOpType.bypass, replica_groups=rg,
                ins=[y1_sh[:]], outs=[y1_full[:]],
            )
            h1_view = h1_sh.ap().rearrange("(t p) d -> p t d", p=P)
            st = None
            for t, ps, y_own in phase_b_tiles(y1_full, y1_sh):
                if t % 4 == 0:
                    st = stage_pool.tile([P, 4, D], f16, tag="stage")
                nc.vector.tensor_add(out=st[:, t % 4, :], in0=ps[:], in1=b_sb[0][:])
                nc.vector.tensor_add(
                    out=st[:, t % 4, :], in0=st[:, t % 4, :], in1=y_own[:]
                )
                if t % 4 == 3:
                    g0 = t - 3
                    nc.sync.dma_start(
                        out=h1_view[:, g0 : g0 + 4, :], in_=st[:]
                    )

            # ---- layer 2 ----
            phase_a(h1_sh, w_sb[1], y2_sh)
            nc.gpsimd.collective_compute(
                "AllGather", mybir.AluOpType.bypass, replica_groups=rg,
                ins=[y2_sh[:]], outs=[y2_full[:]],
            )
            pool_ps = psPool.tile([N_GRAPHS, D], f32)
            for t, ps, y_own in phase_b_tiles(y2_full, y2_sh):
                h2 = h2_pool.tile([P, D], f16, tag="h2")
                nc.vector.tensor_add(out=h2[:], in0=ps[:], in1=b_sb[1][:])
                nc.vector.tensor_add(out=h2[:], in0=h2[:], in1=y_own[:])
                nc.tensor.matmul(
                    out=pool_ps[:],
                    lhsT=p_sb[:, t, :],
                    rhs=h2[:],
                    start=(t == 0),
                    stop=(t == TILES - 1),
                    skip_group_check=True,
                )

            # ---- pooled AllReduce ----
            pool_sb = mlp_pool.tile([N_GRAPHS, D], f32)
            nc.vector.tensor_copy(out=pool_sb[:], in_=pool_ps[:])
            nc.sync.dma_start(out=pool_in[:], in_=pool_sb[:])
            nc.gpsimd.collective_compute(
                "AllReduce", mybir.AluOpType.add, replica_groups=rg,
                ins=[pool_in[:]], outs=[pool_out[:]],
            )

            # ---- classifier MLP (replicated, all f32) ----
            pooled = mlp_pool.tile([N_GRAPHS, D], f32)
            nc.sync.dma_start(out=pooled[:], in_=pool_out[:])
            pooledT = mlp_pool.tile([P, KCH, N_GRAPHS], f32)
            for j in range(KCH):
                ps_t = psMLP.tile([P, N_GRAPHS], f32, tag="psT")
                nc.tensor.transpose(
                    out=ps_t[:],
                    in_=pooled[:, j * P : (j + 1) * P],
                    identity=ident[0:N_GRAPHS, 0:N_GRAPHS],
                )
                nc.vector.tensor_copy(out=pooledT[:, j, :], in_=ps_t[:])
            zT = mlp_pool.tile([P, 2, N_GRAPHS], f32)
            for c2 in range(2):
                ps_z = psMLP.tile([P, N_GRAPHS], f32, tag="psT")
                for j in range(KCH):
                    nc.tensor.matmul(
                        out=ps_z[:],
                        lhsT=wc1_v[:, j, c2, :],
                        rhs=pooledT[:, j, :],
                        start=(j == 0),
                        stop=(j == KCH - 1),
                    )
                nc.scalar.activation(
                    out=zT[:, c2, :], in_=ps_z[:],
                    func=mybir.ActivationFunctionType.Relu,
                    bias=bc1_sb[:, c2 : c2 + 1],
                )
            ps_s = psMLP.tile([1, N_GRAPHS], f32, tag="psS")
            for c2 in range(2):
                nc.tensor.matmul(
                    out=ps_s[:],
                    lhsT=wc2_sb[:, c2 : c2 + 1],
                    rhs=zT[:, c2, :],
                    start=(c2 == 0),
                    stop=(c2 == 1),
                )
            score_sb = mlp_pool.tile([1, N_GRAPHS], f32)
            nc.scalar.activation(
                out=score_sb[:], in_=ps_s[:],
                func=mybir.ActivationFunctionType.Sigmoid,
                bias=bc2_sb[0:1, 0:1],
            )
            nc.sync.dma_start(out=scores[:], in_=score_sb[:])

    nc.finalize()
    return nc


def _wrap_idx(block):
    """[n] -> [16, n/16] wrapped: element i at [i%16, i//16]."""
    n = block.shape[0]
    return block.reshape(n // 16, 16).T


def _prep_inputs(joint_x, joint_edge_index, joint_batch,
                 W_g1, b_g1, W_g2, b_g2, W_c1, b_c1, W_c2, b_c2):
    import heapq

    x = np.asarray(joint_x, np.float32)
    ei = np.asarray(joint_edge_index).astype(np.int64)
    batch = np.asarray(joint_batch).astype(np.int64)
    src, dst = ei[0], ei[1]

    # Unique (src,dst) pairs; multiplicity rides in the S matrix (exact small
    # ints in fp16). Self term (I+A diagonal) is handled separately on-device
    # via a contiguous load of the tile's own y rows, so no self-loop edges.
    pk = src * N_NODES + dst
    upair, mult = np.unique(pk, return_counts=True)
    u_src = upair // N_NODES
    u_dst = upair % N_NODES

    # Rebalance: assign dst nodes to the 160 (core,tile) bins, greedily
    # equalizing per-bin in-edge counts, so every tile needs the same (and
    # minimal) number of 128-edge chunks. The node->position permutation is
    # free to choose: pooling only needs each node's graph id.
    indeg = np.bincount(u_dst, minlength=N_NODES)
    n_bins = N_CORES * TILES
    order = np.argsort(-indeg, kind="stable")
    heap = [(0, b) for b in range(n_bins)]
    heapq.heapify(heap)
    cap = np.full(n_bins, P, np.int64)
    node_bin = np.empty(N_NODES, np.int64)
    node_slot = np.empty(N_NODES, np.int64)
    for n in order:
        while True:
            load, b = heapq.heappop(heap)
            if cap[b] > 0:
                break
        node_bin[n] = b
        node_slot[n] = P - cap[b]
        cap[b] -= 1
        heapq.heappush(heap, (load + int(indeg[n]), b))
    pos = (node_bin // TILES) * PAD_ROWS + (node_bin % TILES) * P + node_slot

    # Gather rows: one per unique (dst-bin, src) — a single gathered y row
    # feeds every dst slot of that tile that has an edge from src.
    bin_of_pair = node_bin[u_dst]
    rk = bin_of_pair * FULL_PAD + pos[u_src]
    urow, row_inv = np.unique(rk, return_inverse=True)
    row_bin = urow // FULL_PAD
    row_psrc = urow % FULL_PAD
    rows_per_bin = np.bincount(row_bin, minlength=n_bins)
    k_max = int((rows_per_bin.max() + P - 1) // P)
    sizes = _gather_split(k_max)

    # Rank within bin (urow sorted => grouped by bin, ascending src pos),
    # then deal 16 ways so each SDMA engine (descriptor i -> engine i%16)
    # walks ascending HBM addresses.
    bin_starts = np.concatenate([[0], np.cumsum(rows_per_bin)])
    row_rank = np.arange(len(urow)) - bin_starts[row_bin]
    n_b = rows_per_bin[row_bin]
    sub_len = (n_b + 15) // 16
    row_pos = (row_rank % sub_len) * 16 + row_rank // sub_len

    per_core = []
    pair_bin = bin_of_pair
    pair_rowpos = row_pos[row_inv]
    pair_slot = node_slot[u_dst]
    for c in range(N_CORES):
        idx_flat = np.zeros((TILES, k_max * P), np.int16)
        m = row_bin // TILES == c
        t = row_bin[m] % TILES
        idx_flat[t, row_pos[m]] = row_psrc[m].astype(np.int16)
        # S: [TILES*k_max*128 rows, 128 slots]
        S = np.zeros((TILES * k_max * P, P), F16)
        pm = pair_bin // TILES == c
        pt = pair_bin[pm] % TILES
        srow = pt * (k_max * P) + pair_rowpos[pm]
        S[srow, pair_slot[pm]] = mult[pm]
        per_core.append((idx_flat, S))

    # node at each padded position (for x shard + pooling construction)
    node_at = np.full(N_CORES * PAD_ROWS, -1, np.int64)
    node_at[pos] = np.arange(N_NODES)

    in_maps = []
    w1_pack = np.ascontiguousarray(
        W_g1.astype(F16).reshape(KCH, P, D).transpose(1, 0, 2).reshape(P, KCH * D))
    w2_pack = np.ascontiguousarray(
        W_g2.astype(F16).reshape(KCH, P, D).transpose(1, 0, 2).reshape(P, KCH * D))
    b1_pack = np.ascontiguousarray(np.broadcast_to(
        np.asarray(b_g1, np.float32), (P, D)))
    b2_pack = np.ascontiguousarray(np.broadcast_to(
        np.asarray(b_g2, np.float32), (P, D)))
    wc1_pack = np.ascontiguousarray(
        np.asarray(W_c1, np.float32).reshape(KCH, P, 2, P)
        .transpose(1, 0, 2, 3).reshape(P, KCH * 2 * P))
    bc1_pack = np.ascontiguousarray(np.asarray(b_c1, np.float32).reshape(2, P).T)
    wc2_pack = np.ascontiguousarray(np.asarray(W_c2, np.float32).reshape(2, P).T)
    bc2_pack = np.asarray(b_c2, np.float32).reshape(1, 1)

    x_bf = x.astype(F16)
    for c in range(N_CORES):
        idx_flat, S = per_core[c]

        # x shard in permuted position space
        nodes_c = node_at[c * PAD_ROWS : (c + 1) * PAD_ROWS]
        real = nodes_c >= 0
        xs = np.zeros((PAD_ROWS, D), F16)
        xs[real] = x_bf[nodes_c[real]]

        s_pack = np.ascontiguousarray(
            S.reshape(TILES * k_max, P, P).transpose(1, 0, 2).reshape(P, -1))

        # gather idx table [128, TILES*k_max*8] wrapped per call
        cols = []
        for t in range(TILES):
            off = 0
            for sz in sizes:
                block = idx_flat[t, off * P : (off + sz) * P]
                cols.append(_wrap_idx(block))
                off += sz
        idx16 = np.concatenate(cols, axis=1)          # [16, TILES*k_max*8]
        idx_pack = np.ascontiguousarray(np.tile(idx16, (8, 1)))

        # pooling one-hot [128, TILES*64]
        Pm = np.zeros((PAD_ROWS, N_GRAPHS), F16)
        Pm[real, batch[nodes_c[real]]] = 1
        p_pack = np.ascontiguousarray(
            Pm.reshape(TILES, P, N_GRAPHS).transpose(1, 0, 2).reshape(P, -1))

        in_maps.append({
            "x_sh": xs,
            "idx_all": idx_pack,
            "s_all": s_pack,
            "p_all": p_pack,
            "w1": w1_pack, "w2": w2_pack,
            "b1b": b1_pack, "b2b": b2_pack,
            "wc1": wc1_pack, "bc1": bc1_pack,
            "wc2": wc2_pack, "bc2": bc2_pack,
        })
    return k_max, in_maps


def kernel(**inputs):
    global LAST_EXEC_NS, LAST_RESULTS
    k_max, in_maps = _prep_inputs(**inputs)
    if k_max not in _prog_cache:
        _prog_cache[k_max] = _build_program(k_max)
    nc = _prog_cache[k_max]
    trace = os.environ.get("GNN_TRACE", "0") == "1"
    res = run_bass_kernel_spmd(
        nc, in_maps, core_ids=list(range(N_CORES)), trace=trace,
        tmpdir=os.environ.get("GNN_TRACE_DIR") or None,
    )
    LAST_EXEC_NS = getattr(res, "exec_time_ns", None)
    LAST_RESULTS = res
    return np.asarray(res.results[0]["scores"]).reshape(N_GRAPHS).astype(np.float32)


# revision 25
# speedup vs baseline: 1.5461x; 1.0797x over previous
"""CrossEncoderGNN (2x GIN layer + sum-pool + MLP + sigmoid) on 8 trn2 NeuronCores.

Strategy
--------
Math: GIN layer  h' = (h + A h) @ W + b  ==  (I + A) (h @ W) + b   (A acts on
rows, W on columns, so they commute).  Per layer:
  phase A: y = h @ W computed on each core for its 2500-node shard (dense
           matmul, xbar-transpose DMA provides h^T tiles as lhsT).
  AllGather: y shards (f16) -> full padded table [8*2560, 512] on every core.
  phase B: per dst-tile of 128 nodes, dma_gather the y rows of all incident
           edges (dst-sorted, self-loops included) and segment-sum them with a
           one-hot [128e x 128d] matmul into PSUM; add bias.
Pooling (graph segment-sum) is one more one-hot matmul accumulated over the
core's 20 node tiles; partial pooled [64,512] is AllReduced, and the tiny
classifier MLP + sigmoid runs replicated on every core.

Sharding: nodes (and their incident in-edges) are split 8 ways by contiguous
dst ranges: core c owns nodes [2500c, 2500c+2500), padded to 2560 rows so
every core has 20 uniform tiles of 128.
"""

import sys

for _p in ("/opt/trn_rl_repo", "/root/.axon_site/_ro/trn_rl_repo"):
    if _p not in sys.path:
        sys.path.insert(0, _p)

import os
import numpy as np
import ml_dtypes

import concourse.bass as bass
import concourse.bacc as bacc
import concourse.tile as tile
from concourse import mybir
from concourse.bass_utils import run_bass_kernel_spmd
from concourse.masks import make_identity

F16 = np.float16

N_NODES = 20000
N_EDGES = 320000
D = 512
N_GRAPHS = 64
N_CORES = 8
ROWS = N_NODES // N_CORES          # 2500 real rows per core
P = 128
TILES = (ROWS + P - 1) // P        # 20
PAD_ROWS = TILES * P               # 2560 padded rows per core
FULL_PAD = PAD_ROWS * N_CORES      # 20480
KCH = D // P                       # 4 contraction chunks of 128

LAST_EXEC_NS = None
LAST_RESULTS = None

_prog_cache = {}


HALF = PAD_ROWS // 2               # 1280 local rows per AllGather half
FULL_HALF = HALF * N_CORES         # 10240 rows per half table


def _gather_split(k_max):
    """Split k_max chunks into balanced calls of <=8 chunks each."""
    n_calls = max(1, (k_max + 7) // 8)
    base = k_max // n_calls
    rem = k_max - base * n_calls
    return [base + (1 if i < rem else 0) for i in range(n_calls)]


def _build_program(k_a, k_b):
    # Each layer's AllGather is split in two halves (local rows [0:1280) and
    # [1280:2560)); phase B's per-tile gathers are likewise split by src half
    # so the first half's gathers overlap the second half's AllGather.
    k_max = k_a + k_b
    sizes_a = _gather_split(k_a)
    sizes_b = _gather_split(k_b)
    kg_max = max(sizes_a + sizes_b)
    f32 = mybir.dt.float32
    f16 = mybir.dt.float16
    i16 = mybir.dt.int16

    nc = bacc.Bacc("TRN2", debug=False, num_devices=N_CORES, num_swdge_queues=4)

    # ---- I/O ----
    x_sh = nc.dram_tensor("x_sh", [PAD_ROWS, D], f16, kind="ExternalInput")
    idx_all = nc.dram_tensor("idx_all", [P, TILES * k_max * 8], i16, kind="ExternalInput")
    s_all = nc.dram_tensor("s_all", [P, TILES * k_max * P], f16, kind="ExternalInput")
    p_all = nc.dram_tensor("p_all", [P, TILES * N_GRAPHS], f16, kind="ExternalInput")
    w1_in = nc.dram_tensor("w1", [P, KCH * D], f16, kind="ExternalInput")
    w2_in = nc.dram_tensor("w2", [P, KCH * D], f16, kind="ExternalInput")
    b1_in = nc.dram_tensor("b1b", [P, D], f32, kind="ExternalInput")
    b2_in = nc.dram_tensor("b2b", [P, D], f32, kind="ExternalInput")
    wc1_in = nc.dram_tensor("wc1", [P, KCH * 2 * P], f32, kind="ExternalInput")
    bc1_in = nc.dram_tensor("bc1", [P, 2], f32, kind="ExternalInput")
    wc2_in = nc.dram_tensor("wc2", [P, 2], f32, kind="ExternalInput")
    bc2_in = nc.dram_tensor("bc2", [1, 1], f32, kind="ExternalInput")
    scores = nc.dram_tensor("scores", [1, N_GRAPHS], f32, kind="ExternalOutput")

    # ---- internal DRAM ----
    y1_sh = nc.dram_tensor("y1_sh", [PAD_ROWS, D], f16)
    y2_sh = nc.dram_tensor("y2_sh", [PAD_ROWS, D], f16)
    h1_sh = nc.dram_tensor("h1_sh", [PAD_ROWS, D], f16)
    y1_fa = nc.dram_tensor("y1_fa", [FULL_HALF, D], f16, addr_space="Shared")
    y1_fb = nc.dram_tensor("y1_fb", [FULL_HALF, D], f16, addr_space="Shared")
    y2_fa = nc.dram_tensor("y2_fa", [FULL_HALF, D], f16, addr_space="Shared")
    y2_fb = nc.dram_tensor("y2_fb", [FULL_HALF, D], f16, addr_space="Shared")
    pool_in = nc.dram_tensor("pool_in", [N_GRAPHS, D], f32)
    pool_out = nc.dram_tensor("pool_out", [N_GRAPHS, D], f32, addr_space="Shared")

    rg = [list(range(N_CORES))]

    with tile.TileContext(nc) as tc:
        with (
            tc.tile_pool(name="const", bufs=1) as const,
            tc.tile_pool(name="xT", bufs=1) as xT_pool,
            tc.tile_pool(name="gbuf", bufs=6) as gpool,
            tc.tile_pool(name="stage", bufs=3) as stage_pool,
            tc.tile_pool(name="h2p", bufs=4) as h2_pool,
            tc.tile_pool(name="mlp", bufs=1) as mlp_pool,
            tc.tile_pool(name="psA", bufs=4, space="PSUM") as psA,
            tc.tile_pool(name="psPool", bufs=1, space="PSUM") as psPool,
            tc.tile_pool(name="psMLP", bufs=1, space="PSUM") as psMLP,
        ):
            # ---- resident constants ----
            # Bulk loads go through the ACT HWDGE ring (nc.scalar) so they
            # don't serialize with phase A's xbar transposes on the SP ring.
            idx_sb = const.tile([P, TILES * k_max * 8], i16)
            nc.scalar.dma_start(out=idx_sb[:], in_=idx_all[:])
            s_flat = const.tile([P, TILES * k_max * P], f16)
            nc.scalar.dma_start(out=s_flat[:], in_=s_all[:])
            s_sb = s_flat[:].rearrange("p (c d) -> p c d", d=P)
            p_flat = const.tile([P, TILES * N_GRAPHS], f16)
            nc.scalar.dma_start(out=p_flat[:], in_=p_all[:])
            p_sb = p_flat[:].rearrange("p (t g) -> p t g", g=N_GRAPHS)
            w_sb = []
            for w_in in (w1_in, w2_in):
                wt = const.tile([P, KCH * D], f16)
                nc.scalar.dma_start(out=wt[:], in_=w_in[:])
                w_sb.append(wt[:].rearrange("p (j d) -> p j d", d=D))
            b_sb = []
            for b_in in (b1_in, b2_in):
                bt = const.tile([P, D], f32)
                nc.scalar.dma_start(out=bt[:], in_=b_in[:])
                b_sb.append(bt)
            wc1_sb = const.tile([P, KCH * 2 * P], f32)
            nc.scalar.dma_start(out=wc1_sb[:], in_=wc1_in[:])
            wc1_v = wc1_sb[:].rearrange("p (j c m) -> p j c m", c=2, m=P)
            bc1_sb = const.tile([P, 2], f32)
            nc.scalar.dma_start(out=bc1_sb[:], in_=bc1_in[:])
            wc2_sb = const.tile([P, 2], f32)
            nc.scalar.dma_start(out=wc2_sb[:], in_=wc2_in[:])
            bc2_sb = const.tile([1, 1], f32)
            nc.scalar.dma_start(out=bc2_sb[:], in_=bc2_in[:])
            ident = const.tile([P, P], f32)
            make_identity(nc, ident[:])

            def phase_a(h_dram, w_view, y_dram):
                """y = h @ W for this core's 20 row tiles."""
                xT = xT_pool.tile([P, KCH, PAD_ROWS], f16, tag="xT")
                for j in range(KCH):
                    nc.sync.dma_start(
                        out=xT[:, j, :],
                        in_=h_dram[:, j * P : (j + 1) * P],
                        transpose=True,
                    )
                y_view = y_dram.ap().rearrange("(t p) d -> p t d", p=P)
                for grp in range(TILES // 4):
                    st = stage_pool.tile([P, 4, D], f16, tag="stage")
                    for tt in range(4):
                        t = grp * 4 + tt
                        ps = psA.tile([P, D], f32, tag="psA")
                        for j in range(KCH):
                            nc.tensor.matmul(
                                out=ps[:],
                                lhsT=xT[:, j, t * P : (t + 1) * P],
                                rhs=w_view[:, j, :],
                                start=(j == 0),
                                stop=(j == KCH - 1),
                            )
                        nc.vector.tensor_copy(out=st[:, tt, :], in_=ps[:])
                    nc.sync.dma_start(
                        out=y_view[:, grp * 4 : grp * 4 + 4, :], in_=st[:]
                    )

            qn_counter = [0]

            def phase_b_tiles(y_fa, y_fb, y_sh):
                """Yield (t, agg_psum, y_own) for each dst tile.

                y_own is the tile's own 128 rows of y, loaded contiguously
                from the local shard (the GIN self term), to be DVE-added by
                the consumer along with the bias.
                """
                for t in range(TILES):
                    y_own = h2_pool.tile([P, D], f16, tag="yown")
                    nc.sync.dma_start(
                        out=y_own[:], in_=y_sh[t * P : (t + 1) * P, :]
                    )
                    gs = []
                    off = 0
                    for y_half, szs in ((y_fa, sizes_a), (y_fb, sizes_b)):
                        for sz in szs:
                            g = gpool.tile([P, kg_max, D], f16, tag="g")
                            col0 = (t * k_max + off) * 8
                            nc.gpsimd.dma_gather(
                                out_ap=g[:, :sz, :],
                                in_ap=y_half[:],
                                idxs_ap=idx_sb[:, col0 : col0 + sz * 8],
                                num_idxs=sz * P,
                                num_idxs_reg=sz * P,
                                elem_size=D,
                                queue_num=qn_counter[0] % 4,
                            )
                            qn_counter[0] += 1
                            gs.append((g, off, sz))
                            off += sz
                    ps = psA.tile([P, D], f32, tag="psA")
                    ki = 0
                    for g, off, sz in gs:
                        for k in range(sz):
                            nc.tensor.matmul(
                                out=ps[:],
                                lhsT=s_sb[:, t * k_max + off + k, :],
                                rhs=g[:, k, :],
                                start=(ki == 0),
                                stop=(ki == k_max - 1),
                            )
                            ki += 1
                    yield t, ps, y_own

            def allgather_halves(y_sh, y_fa, y_fb):
                nc.gpsimd.collective_compute(
                    "AllGather", mybir.AluOpType.bypass, replica_groups=rg,
                    ins=[y_sh[0:HALF, :]], outs=[y_fa[:]],
                )
                nc.gpsimd.collective_compute(
                    "AllGather", mybir.AluOpType.bypass, replica_groups=rg,
                    ins=[y_sh[HALF:PAD_ROWS, :]], outs=[y_fb[:]],
                )

            # ---- layer 1 ----
            phase_a(x_sh, w_sb[0], y1_sh)
            allgather_halves(y1_sh, y1_fa, y1_fb)
            h1_view = h1_sh.ap().rearrange("(t p) d -> p t d", p=P)
            st = None
            for t, ps, y_own in phase_b_tiles(y1_fa, y1_fb, y1_sh):
                if t % 4 == 0:
                    st = stage_pool.tile([P, 4, D], f16, tag="stage")
                nc.vector.tensor_add(out=st[:, t % 4, :], in0=ps[:], in1=b_sb[0][:])
                nc.vector.tensor_add(
                    out=st[:, t % 4, :], in0=st[:, t % 4, :], in1=y_own[:]
                )
                if t % 4 == 3:
                    g0 = t - 3
                    nc.sync.dma_start(
                        out=h1_view[:, g0 : g0 + 4, :], in_=st[:]
                    )

            # ---- layer 2 ----
            phase_a(h1_sh, w_sb[1], y2_sh)
            allgather_halves(y2_sh, y2_fa, y2_fb)
            pool_ps = psPool.tile([N_GRAPHS, D], f32)
            for t, ps, y_own in phase_b_tiles(y2_fa, y2_fb, y2_sh):
                h2 = h2_pool.tile([P, D], f16, tag="h2")
                nc.vector.tensor_add(out=h2[:], in0=ps[:], in1=b_sb[1][:])
                nc.vector.tensor_add(out=h2[:], in0=h2[:], in1=y_own[:])
                nc.tensor.matmul(
                    out=pool_ps[:],
                    lhsT=p_sb[:, t, :],
                    rhs=h2[:],
                    start=(t == 0),
                    stop=(t == TILES - 1),
                    skip_group_check=True,
                )

            # ---- pooled AllReduce ----
            pool_sb = mlp_pool.tile([N_GRAPHS, D], f32)
            nc.vector.tensor_copy(out=pool_sb[:], in_=pool_ps[:])
            nc.sync.dma_start(out=pool_in[:], in_=pool_sb[:])
            nc.gpsimd.collective_compute(
                "AllReduce", mybir.AluOpType.add, replica_groups=rg,
                ins=[pool_in[:]], outs=[pool_out[:]],
            )

            # ---- classifier MLP (replicated, all f32) ----
            pooled = mlp_pool.tile([N_GRAPHS, D], f32)
            nc.sync.dma_start(out=pooled[:], in_=pool_out[:])
            pooledT = mlp_pool.tile([P, KCH, N_GRAPHS], f32)
            for j in range(KCH):
                ps_t = psMLP.tile([P, N_GRAPHS], f32, tag="psT")
                nc.tensor.transpose(
                    out=ps_t[:],
                    in_=pooled[:, j * P : (j + 1) * P],
                    identity=ident[0:N_GRAPHS, 0:N_GRAPHS],
                )
                nc.vector.tensor_copy(out=pooledT[:, j, :], in_=ps_t[:])
            zT = mlp_pool.tile([P, 2, N_GRAPHS], f32)
            for c2 in range(2):
                ps_z = psMLP.tile([P, N_GRAPHS], f32, tag="psT")
                for j in range(KCH):
                    nc.tensor.matmul(
                        out=ps_z[:],
                        lhsT=wc1_v[:, j, c2, :],
                        rhs=pooledT[:, j, :],
                        start=(j == 0),
                        stop=(j == KCH - 1),
                    )
                nc.scalar.activation(
                    out=zT[:, c2, :], in_=ps_z[:],
                    func=mybir.ActivationFunctionType.Relu,
                    bias=bc1_sb[:, c2 : c2 + 1],
                )
            ps_s = psMLP.tile([1, N_GRAPHS], f32, tag="psS")
            for c2 in range(2):
                nc.tensor.matmul(
                    out=ps_s[:],
                    lhsT=wc2_sb[:, c2 : c2 + 1],
                    rhs=zT[:, c2, :],
                    start=(c2 == 0),
                    stop=(c2 == 1),
                )
            score_sb = mlp_pool.tile([1, N_GRAPHS], f32)
            nc.scalar.activation(
                out=score_sb[:], in_=ps_s[:],
                func=mybir.ActivationFunctionType.Sigmoid,
                bias=bc2_sb[0:1, 0:1],
            )
            nc.sync.dma_start(out=scores[:], in_=score_sb[:])

    nc.finalize()
    return nc


def _wrap_idx(block):
    """[n] -> [16, n/16] wrapped: element i at [i%16, i//16]."""
    n = block.shape[0]
    return block.reshape(n // 16, 16).T


def _prep_inputs(joint_x, joint_edge_index, joint_batch,
                 W_g1, b_g1, W_g2, b_g2, W_c1, b_c1, W_c2, b_c2):
    import heapq

    x = np.asarray(joint_x, np.float32)
    ei = np.asarray(joint_edge_index).astype(np.int64)
    batch = np.asarray(joint_batch).astype(np.int64)
    src, dst = ei[0], ei[1]

    # Unique (src,dst) pairs; multiplicity rides in the S matrix (exact small
    # ints in fp16). Self term (I+A diagonal) is handled separately on-device
    # via a contiguous load of the tile's own y rows, so no self-loop edges.
    pk = src * N_NODES + dst
    upair, mult = np.unique(pk, return_counts=True)
    u_src = upair // N_NODES
    u_dst = upair % N_NODES

    # Rebalance: assign dst nodes to the 160 (core,tile) bins, greedily
    # equalizing per-bin in-edge counts, so every tile needs the same (and
    # minimal) number of 128-edge chunks. The node->position permutation is
    # free to choose: pooling only needs each node's graph id.
    indeg = np.bincount(u_dst, minlength=N_NODES)
    n_bins = N_CORES * TILES
    order = np.argsort(-indeg, kind="stable")
    heap = [(0, b) for b in range(n_bins)]
    heapq.heapify(heap)
    cap = np.full(n_bins, P, np.int64)
    node_bin = np.empty(N_NODES, np.int64)
    node_slot = np.empty(N_NODES, np.int64)
    for n in order:
        while True:
            load, b = heapq.heappop(heap)
            if cap[b] > 0:
                break
        node_bin[n] = b
        node_slot[n] = P - cap[b]
        cap[b] -= 1
        heapq.heappush(heap, (load + int(indeg[n]), b))
    pos = (node_bin // TILES) * PAD_ROWS + (node_bin % TILES) * P + node_slot

    # Gather rows: one per unique (dst-bin, src-half, src) — a single
    # gathered y row feeds every dst slot of that tile that has an edge from
    # src. Rows are split by src HALF (local row </>= 1280) so each tile's
    # first gather call only depends on the first AllGather half.
    bin_of_pair = node_bin[u_dst]
    src_pos = pos[u_src]
    src_half = (src_pos % PAD_ROWS) // HALF
    src_hidx = (src_pos // PAD_ROWS) * HALF + (src_pos % PAD_ROWS) % HALF
    rk = (bin_of_pair * 2 + src_half) * FULL_HALF + src_hidx
    urow, row_inv = np.unique(rk, return_inverse=True)
    row_bh = urow // FULL_HALF
    row_psrc = urow % FULL_HALF          # index into the half table
    rows_per_bh = np.bincount(row_bh, minlength=n_bins * 2)
    k_a = int((rows_per_bh[0::2].max() + P - 1) // P)
    k_b = int((rows_per_bh[1::2].max() + P - 1) // P)
    k_max = k_a + k_b
    sizes = _gather_split(k_a) + _gather_split(k_b)

    # Rank within (bin, half) (urow sorted => grouped, ascending src pos),
    # then deal 16 ways so each SDMA engine (descriptor i -> engine i%16)
    # walks ascending HBM addresses.
    bh_starts = np.concatenate([[0], np.cumsum(rows_per_bh)])
    row_rank = np.arange(len(urow)) - bh_starts[row_bh]
    n_b = rows_per_bh[row_bh]
    sub_len = (n_b + 15) // 16
    pos_in_half = (row_rank % sub_len) * 16 + row_rank // sub_len
    row_pos = np.where(row_bh % 2 == 0, pos_in_half, k_a * P + pos_in_half)
    row_bin = row_bh // 2

    per_core = []
    pair_bin = bin_of_pair
    pair_rowpos = row_pos[row_inv]
    pair_slot = node_slot[u_dst]
    for c in range(N_CORES):
        idx_flat = np.zeros((TILES, k_max * P), np.int16)
        m = row_bin // TILES == c
        t = row_bin[m] % TILES
        idx_flat[t, row_pos[m]] = row_psrc[m].astype(np.int16)
        # S: [TILES*k_max*128 rows, 128 slots]
        S = np.zeros((TILES * k_max * P, P), F16)
        pm = pair_bin // TILES == c
        pt = pair_bin[pm] % TILES
        srow = pt * (k_max * P) + pair_rowpos[pm]
        S[srow, pair_slot[pm]] = mult[pm]
        per_core.append((idx_flat, S))

    # node at each padded position (for x shard + pooling construction)
    node_at = np.full(N_CORES * PAD_ROWS, -1, np.int64)
    node_at[pos] = np.arange(N_NODES)

    in_maps = []
    w1_pack = np.ascontiguousarray(
        W_g1.astype(F16).reshape(KCH, P, D).transpose(1, 0, 2).reshape(P, KCH * D))
    w2_pack = np.ascontiguousarray(
        W_g2.astype(F16).reshape(KCH, P, D).transpose(1, 0, 2).reshape(P, KCH * D))
    b1_pack = np.ascontiguousarray(np.broadcast_to(
        np.asarray(b_g1, np.float32), (P, D)))
    b2_pack = np.ascontiguousarray(np.broadcast_to(
        np.asarray(b_g2, np.float32), (P, D)))
    wc1_pack = np.ascontiguousarray(
        np.asarray(W_c1, np.float32).reshape(KCH, P, 2, P)
        .transpose(1, 0, 2, 3).reshape(P, KCH * 2 * P))
    bc1_pack = np.ascontiguousarray(np.asarray(b_c1, np.float32).reshape(2, P).T)
    wc2_pack = np.ascontiguousarray(np.asarray(W_c2, np.float32).reshape(2, P).T)
    bc2_pack = np.asarray(b_c2, np.float32).reshape(1, 1)

    x_bf = x.astype(F16)
    for c in range(N_CORES):
        idx_flat, S = per_core[c]

        # x shard in permuted position space
        nodes_c = node_at[c * PAD_ROWS : (c + 1) * PAD_ROWS]
        real = nodes_c >= 0
        xs = np.zeros((PAD_ROWS, D), F16)
        xs[real] = x_bf[nodes_c[real]]

        s_pack = np.ascontiguousarray(
            S.reshape(TILES * k_max, P, P).transpose(1, 0, 2).reshape(P, -1))

        # gather idx table [128, TILES*k_max*8] wrapped per call
        cols = []
        for t in range(TILES):
            off = 0
            for sz in sizes:
                block = idx_flat[t, off * P : (off + sz) * P]
                cols.append(_wrap_idx(block))
                off += sz
        idx16 = np.concatenate(cols, axis=1)          # [16, TILES*k_max*8]
        idx_pack = np.ascontiguousarray(np.tile(idx16, (8, 1)))

        # pooling one-hot [128, TILES*64]
        Pm = np.zeros((PAD_ROWS, N_GRAPHS), F16)
        Pm[real, batch[nodes_c[real]]] = 1
        p_pack = np.ascontiguousarray(
            Pm.reshape(TILES, P, N_GRAPHS).transpose(1, 0, 2).reshape(P, -1))

        in_maps.append({
            "x_sh": xs,
            "idx_all": idx_pack,
            "s_all": s_pack,
            "p_all": p_pack,
            "w1": w1_pack, "w2": w2_pack,
            "b1b": b1_pack, "b2b": b2_pack,
            "wc1": wc1_pack, "bc1": bc1_pack,
            "wc2": wc2_pack, "bc2": bc2_pack,
        })
    return (k_a, k_b), in_maps


def kernel(**inputs):
    global LAST_EXEC_NS, LAST_RESULTS
    kk, in_maps = _prep_inputs(**inputs)
    if kk not in _prog_cache:
        _prog_cache[kk] = _build_program(*kk)
    nc = _prog_cache[kk]
    trace = os.environ.get("GNN_TRACE", "0") == "1"
    res = run_bass_kernel_spmd(
        nc, in_maps, core_ids=list(range(N_CORES)), trace=trace,
        tmpdir=os.environ.get("GNN_TRACE_DIR") or None,
    )
    LAST_EXEC_NS = getattr(res, "exec_time_ns", None)
    LAST_RESULTS = res
    return np.asarray(res.results[0]["scores"]).reshape(N_GRAPHS).astype(np.float32)


# revision 28
# speedup vs baseline: 1.5813x; 1.0228x over previous
"""CrossEncoderGNN (2x GIN layer + sum-pool + MLP + sigmoid) on 8 trn2 NeuronCores.

Strategy
--------
Math: GIN layer  h' = (h + A h) @ W + b  ==  (I + A) (h @ W) + b   (A acts on
rows, W on columns, so they commute).  Per layer:
  phase A: y = h @ W computed on each core for its 2500-node shard (dense
           matmul, xbar-transpose DMA provides h^T tiles as lhsT).
  AllGather: y shards (f16) -> full padded table [8*2560, 512] on every core.
  phase B: per dst-tile of 128 nodes, dma_gather the y rows of all incident
           edges (dst-sorted, self-loops included) and segment-sum them with a
           one-hot [128e x 128d] matmul into PSUM; add bias.
Pooling (graph segment-sum) is one more one-hot matmul accumulated over the
core's 20 node tiles; partial pooled [64,512] is AllReduced, and the tiny
classifier MLP + sigmoid runs replicated on every core.

Sharding: nodes (and their incident in-edges) are split 8 ways by contiguous
dst ranges: core c owns nodes [2500c, 2500c+2500), padded to 2560 rows so
every core has 20 uniform tiles of 128.
"""

import sys

for _p in ("/opt/trn_rl_repo", "/root/.axon_site/_ro/trn_rl_repo"):
    if _p not in sys.path:
        sys.path.insert(0, _p)

import os
import numpy as np
import ml_dtypes

import concourse.bass as bass
import concourse.bacc as bacc
import concourse.tile as tile
from concourse import mybir
from concourse.bass_utils import run_bass_kernel_spmd
from concourse.masks import make_identity

F16 = np.float16

N_NODES = 20000
N_EDGES = 320000
D = 512
N_GRAPHS = 64
N_CORES = 8
ROWS = N_NODES // N_CORES          # 2500 real rows per core
P = 128
TILES = (ROWS + P - 1) // P        # 20
PAD_ROWS = TILES * P               # 2560 padded rows per core
FULL_PAD = PAD_ROWS * N_CORES      # 20480
KCH = D // P                       # 4 contraction chunks of 128

LAST_EXEC_NS = None
LAST_RESULTS = None

_prog_cache = {}


HALF = PAD_ROWS // 2               # 1280 local rows per AllGather half
FULL_HALF = HALF * N_CORES         # 10240 rows per half table


def _gather_split(k_max):
    """Split k_max chunks into balanced calls of <=8 chunks each."""
    n_calls = max(1, (k_max + 7) // 8)
    base = k_max // n_calls
    rem = k_max - base * n_calls
    return [base + (1 if i < rem else 0) for i in range(n_calls)]


def _build_program(k_a, k_b):
    # Each layer's AllGather is split in two halves (local rows [0:1280) and
    # [1280:2560)); phase B's per-tile gathers are likewise split by src half
    # so the first half's gathers overlap the second half's AllGather.
    k_max = k_a + k_b
    sizes_a = _gather_split(k_a)
    sizes_b = _gather_split(k_b)
    kg_max = max(sizes_a + sizes_b)
    f32 = mybir.dt.float32
    f16 = mybir.dt.float16
    i16 = mybir.dt.int16

    nc = bacc.Bacc("TRN2", debug=False, num_devices=N_CORES, num_swdge_queues=4)

    # ---- I/O ----
    x_sh = nc.dram_tensor("x_sh", [PAD_ROWS, D], f16, kind="ExternalInput")
    idx_all = nc.dram_tensor("idx_all", [P, TILES * k_max * 8], i16, kind="ExternalInput")
    s_all = nc.dram_tensor("s_all", [P, TILES * k_max * P], f16, kind="ExternalInput")
    p_all = nc.dram_tensor("p_all", [P, TILES * N_GRAPHS], f16, kind="ExternalInput")
    w1_in = nc.dram_tensor("w1", [P, KCH * D], f16, kind="ExternalInput")
    w2_in = nc.dram_tensor("w2", [P, KCH * D], f16, kind="ExternalInput")
    b1_in = nc.dram_tensor("b1b", [P, D], f32, kind="ExternalInput")
    b2_in = nc.dram_tensor("b2b", [P, D], f32, kind="ExternalInput")
    wc1_in = nc.dram_tensor("wc1", [P, KCH * 2 * P], f32, kind="ExternalInput")
    bc1_in = nc.dram_tensor("bc1", [P, 2], f32, kind="ExternalInput")
    wc2_in = nc.dram_tensor("wc2", [P, 2], f32, kind="ExternalInput")
    bc2_in = nc.dram_tensor("bc2", [1, 1], f32, kind="ExternalInput")
    scores = nc.dram_tensor("scores", [1, N_GRAPHS], f32, kind="ExternalOutput")

    # ---- internal DRAM ----
    y1_sh = nc.dram_tensor("y1_sh", [PAD_ROWS, D], f16)
    y2_sh = nc.dram_tensor("y2_sh", [PAD_ROWS, D], f16)
    h1_sh = nc.dram_tensor("h1_sh", [PAD_ROWS, D], f16)
    y1_fa = nc.dram_tensor("y1_fa", [FULL_HALF, D], f16, addr_space="Shared")
    y1_fb = nc.dram_tensor("y1_fb", [FULL_HALF, D], f16, addr_space="Shared")
    y2_fa = nc.dram_tensor("y2_fa", [FULL_HALF, D], f16, addr_space="Shared")
    y2_fb = nc.dram_tensor("y2_fb", [FULL_HALF, D], f16, addr_space="Shared")
    pool_in = nc.dram_tensor("pool_in", [N_GRAPHS, D], f32)
    pool_out = nc.dram_tensor("pool_out", [N_GRAPHS, D], f32, addr_space="Shared")

    rg = [list(range(N_CORES))]

    with tile.TileContext(nc) as tc:
        with (
            tc.tile_pool(name="const", bufs=1) as const,
            tc.tile_pool(name="xT", bufs=1) as xT_pool,
            tc.tile_pool(name="gbuf", bufs=6) as gpool,
            tc.tile_pool(name="stage", bufs=3) as stage_pool,
            tc.tile_pool(name="h2p", bufs=4) as h2_pool,
            tc.tile_pool(name="mlp", bufs=1) as mlp_pool,
            tc.tile_pool(name="psA", bufs=4, space="PSUM") as psA,
            tc.tile_pool(name="psPool", bufs=1, space="PSUM") as psPool,
            tc.tile_pool(name="psMLP", bufs=1, space="PSUM") as psMLP,
        ):
            def load_xT(h_dram):
                xT = xT_pool.tile([P, KCH, PAD_ROWS], f16, tag="xT")
                for j in range(KCH):
                    nc.sync.dma_start(
                        out=xT[:, j, :],
                        in_=h_dram[:, j * P : (j + 1) * P],
                        transpose=True,
                    )
                return xT

            # Layer-1 transposes first: xbar-mode DMAs serialize against
            # normal DMAs, so issue all four before any other traffic.
            xT1 = load_xT(x_sh)

            # ---- resident constants ----
            # Bulk loads go through the ACT HWDGE ring (nc.scalar) so they
            # don't serialize with phase A's xbar transposes on the SP ring.
            idx_sb = const.tile([P, TILES * k_max * 8], i16)
            nc.scalar.dma_start(out=idx_sb[:], in_=idx_all[:])
            s_flat = const.tile([P, TILES * k_max * P], f16)
            nc.scalar.dma_start(out=s_flat[:], in_=s_all[:])
            s_sb = s_flat[:].rearrange("p (c d) -> p c d", d=P)
            p_flat = const.tile([P, TILES * N_GRAPHS], f16)
            nc.scalar.dma_start(out=p_flat[:], in_=p_all[:])
            p_sb = p_flat[:].rearrange("p (t g) -> p t g", g=N_GRAPHS)
            w_sb = []
            for w_in in (w1_in, w2_in):
                wt = const.tile([P, KCH * D], f16)
                nc.scalar.dma_start(out=wt[:], in_=w_in[:])
                w_sb.append(wt[:].rearrange("p (j d) -> p j d", d=D))
            b_sb = []
            for b_in in (b1_in, b2_in):
                bt = const.tile([P, D], f32)
                nc.scalar.dma_start(out=bt[:], in_=b_in[:])
                b_sb.append(bt)
            wc1_sb = const.tile([P, KCH * 2 * P], f32)
            nc.scalar.dma_start(out=wc1_sb[:], in_=wc1_in[:])
            wc1_v = wc1_sb[:].rearrange("p (j c m) -> p j c m", c=2, m=P)
            bc1_sb = const.tile([P, 2], f32)
            nc.scalar.dma_start(out=bc1_sb[:], in_=bc1_in[:])
            wc2_sb = const.tile([P, 2], f32)
            nc.scalar.dma_start(out=wc2_sb[:], in_=wc2_in[:])
            bc2_sb = const.tile([1, 1], f32)
            nc.scalar.dma_start(out=bc2_sb[:], in_=bc2_in[:])
            ident = const.tile([P, P], f32)
            make_identity(nc, ident[:])

            def phase_a(h_dram, w_view, y_dram, xT=None):
                """y = h @ W for this core's 20 row tiles."""
                if xT is None:
                    xT = load_xT(h_dram)
                y_view = y_dram.ap().rearrange("(t p) d -> p t d", p=P)
                for grp in range(TILES // 4):
                    st = stage_pool.tile([P, 4, D], f16, tag="stage")
                    for tt in range(4):
                        t = grp * 4 + tt
                        ps = psA.tile([P, D], f32, tag="psA")
                        for j in range(KCH):
                            nc.tensor.matmul(
                                out=ps[:],
                                lhsT=xT[:, j, t * P : (t + 1) * P],
                                rhs=w_view[:, j, :],
                                start=(j == 0),
                                stop=(j == KCH - 1),
                            )
                        nc.vector.tensor_copy(out=st[:, tt, :], in_=ps[:])
                    nc.sync.dma_start(
                        out=y_view[:, grp * 4 : grp * 4 + 4, :], in_=st[:]
                    )

            qn_counter = [0]

            def phase_b_tiles(y_fa, y_fb, y_sh):
                """Yield (t, agg_psum, y_own) for each dst tile.

                y_own is the tile's own 128 rows of y, loaded contiguously
                from the local shard (the GIN self term), to be DVE-added by
                the consumer along with the bias.
                """
                for t in range(TILES):
                    y_own = h2_pool.tile([P, D], f16, tag="yown")
                    nc.sync.dma_start(
                        out=y_own[:], in_=y_sh[t * P : (t + 1) * P, :]
                    )
                    gs = []
                    off = 0
                    for y_half, szs in ((y_fa, sizes_a), (y_fb, sizes_b)):
                        for sz in szs:
                            g = gpool.tile([P, kg_max, D], f16, tag="g")
                            col0 = (t * k_max + off) * 8
                            nc.gpsimd.dma_gather(
                                out_ap=g[:, :sz, :],
                                in_ap=y_half[:],
                                idxs_ap=idx_sb[:, col0 : col0 + sz * 8],
                                num_idxs=sz * P,
                                num_idxs_reg=sz * P,
                                elem_size=D,
                                queue_num=qn_counter[0] % 4,
                            )
                            qn_counter[0] += 1
                            gs.append((g, off, sz))
                            off += sz
                    ps = psA.tile([P, D], f32, tag="psA")
                    ki = 0
                    for g, off, sz in gs:
                        for k in range(sz):
                            nc.tensor.matmul(
                                out=ps[:],
                                lhsT=s_sb[:, t * k_max + off + k, :],
                                rhs=g[:, k, :],
                                start=(ki == 0),
                                stop=(ki == k_max - 1),
                            )
                            ki += 1
                    yield t, ps, y_own

            def allgather_halves(y_sh, y_fa, y_fb):
                nc.gpsimd.collective_compute(
                    "AllGather", mybir.AluOpType.bypass, replica_groups=rg,
                    ins=[y_sh[0:HALF, :]], outs=[y_fa[:]],
                )
                nc.gpsimd.collective_compute(
                    "AllGather", mybir.AluOpType.bypass, replica_groups=rg,
                    ins=[y_sh[HALF:PAD_ROWS, :]], outs=[y_fb[:]],
                )

            # ---- layer 1 ----
            phase_a(x_sh, w_sb[0], y1_sh, xT=xT1)
            allgather_halves(y1_sh, y1_fa, y1_fb)
            h1_view = h1_sh.ap().rearrange("(t p) d -> p t d", p=P)
            st = None
            for t, ps, y_own in phase_b_tiles(y1_fa, y1_fb, y1_sh):
                if t % 4 == 0:
                    st = stage_pool.tile([P, 4, D], f16, tag="stage")
                nc.vector.tensor_add(out=st[:, t % 4, :], in0=ps[:], in1=b_sb[0][:])
                nc.vector.tensor_add(
                    out=st[:, t % 4, :], in0=st[:, t % 4, :], in1=y_own[:]
                )
                if t % 4 == 3:
                    g0 = t - 3
                    nc.sync.dma_start(
                        out=h1_view[:, g0 : g0 + 4, :], in_=st[:]
                    )

            # ---- layer 2 ----
            phase_a(h1_sh, w_sb[1], y2_sh)
            allgather_halves(y2_sh, y2_fa, y2_fb)
            pool_ps = psPool.tile([N_GRAPHS, D], f32)
            for t, ps, y_own in phase_b_tiles(y2_fa, y2_fb, y2_sh):
                h2 = h2_pool.tile([P, D], f16, tag="h2")
                nc.vector.tensor_add(out=h2[:], in0=ps[:], in1=b_sb[1][:])
                nc.vector.tensor_add(out=h2[:], in0=h2[:], in1=y_own[:])
                nc.tensor.matmul(
                    out=pool_ps[:],
                    lhsT=p_sb[:, t, :],
                    rhs=h2[:],
                    start=(t == 0),
                    stop=(t == TILES - 1),
                    skip_group_check=True,
                )

            # ---- pooled AllReduce ----
            pool_sb = mlp_pool.tile([N_GRAPHS, D], f32)
            nc.vector.tensor_copy(out=pool_sb[:], in_=pool_ps[:])
            nc.sync.dma_start(out=pool_in[:], in_=pool_sb[:])
            nc.gpsimd.collective_compute(
                "AllReduce", mybir.AluOpType.add, replica_groups=rg,
                ins=[pool_in[:]], outs=[pool_out[:]],
            )

            # ---- classifier MLP (replicated, all f32) ----
            pooled = mlp_pool.tile([N_GRAPHS, D], f32)
            nc.sync.dma_start(out=pooled[:], in_=pool_out[:])
            pooledT = mlp_pool.tile([P, KCH, N_GRAPHS], f32)
            for j in range(KCH):
                ps_t = psMLP.tile([P, N_GRAPHS], f32, tag="psT")
                nc.tensor.transpose(
                    out=ps_t[:],
                    in_=pooled[:, j * P : (j + 1) * P],
                    identity=ident[0:N_GRAPHS, 0:N_GRAPHS],
                )
                nc.vector.tensor_copy(out=pooledT[:, j, :], in_=ps_t[:])
            zT = mlp_pool.tile([P, 2, N_GRAPHS], f32)
            for c2 in range(2):
                ps_z = psMLP.tile([P, N_GRAPHS], f32, tag="psT")
                for j in range(KCH):
                    nc.tensor.matmul(
                        out=ps_z[:],
                        lhsT=wc1_v[:, j, c2, :],
                        rhs=pooledT[:, j, :],
                        start=(j == 0),
                        stop=(j == KCH - 1),
                    )
                nc.scalar.activation(
                    out=zT[:, c2, :], in_=ps_z[:],
                    func=mybir.ActivationFunctionType.Relu,
                    bias=bc1_sb[:, c2 : c2 + 1],
                )
            ps_s = psMLP.tile([1, N_GRAPHS], f32, tag="psS")
            for c2 in range(2):
                nc.tensor.matmul(
                    out=ps_s[:],
                    lhsT=wc2_sb[:, c2 : c2 + 1],
                    rhs=zT[:, c2, :],
                    start=(c2 == 0),
                    stop=(c2 == 1),
                )
            score_sb = mlp_pool.tile([1, N_GRAPHS], f32)
            nc.scalar.activation(
                out=score_sb[:], in_=ps_s[:],
                func=mybir.ActivationFunctionType.Sigmoid,
                bias=bc2_sb[0:1, 0:1],
            )
            nc.sync.dma_start(out=scores[:], in_=score_sb[:])

    nc.finalize()
    return nc


def _wrap_idx(block):
    """[n] -> [16, n/16] wrapped: element i at [i%16, i//16]."""
    n = block.shape[0]
    return block.reshape(n // 16, 16).T


def _prep_inputs(joint_x, joint_edge_index, joint_batch,
                 W_g1, b_g1, W_g2, b_g2, W_c1, b_c1, W_c2, b_c2):
    import heapq

    x = np.asarray(joint_x, np.float32)
    ei = np.asarray(joint_edge_index).astype(np.int64)
    batch = np.asarray(joint_batch).astype(np.int64)
    src, dst = ei[0], ei[1]

    # Unique (src,dst) pairs; multiplicity rides in the S matrix (exact small
    # ints in fp16). Self term (I+A diagonal) is handled separately on-device
    # via a contiguous load of the tile's own y rows, so no self-loop edges.
    pk = src * N_NODES + dst
    upair, mult = np.unique(pk, return_counts=True)
    u_src = upair // N_NODES
    u_dst = upair % N_NODES

    # Rebalance: assign dst nodes to the 160 (core,tile) bins, greedily
    # equalizing per-bin in-edge counts, so every tile needs the same (and
    # minimal) number of 128-edge chunks. The node->position permutation is
    # free to choose: pooling only needs each node's graph id.
    indeg = np.bincount(u_dst, minlength=N_NODES)
    n_bins = N_CORES * TILES
    order = np.argsort(-indeg, kind="stable")
    heap = [(0, b) for b in range(n_bins)]
    heapq.heapify(heap)
    cap = np.full(n_bins, P, np.int64)
    node_bin = np.empty(N_NODES, np.int64)
    node_slot = np.empty(N_NODES, np.int64)
    for n in order:
        while True:
            load, b = heapq.heappop(heap)
            if cap[b] > 0:
                break
        node_bin[n] = b
        node_slot[n] = P - cap[b]
        cap[b] -= 1
        heapq.heappush(heap, (load + int(indeg[n]), b))
    pos = (node_bin // TILES) * PAD_ROWS + (node_bin % TILES) * P + node_slot

    # Gather rows: one per unique (dst-bin, src-half, src) — a single
    # gathered y row feeds every dst slot of that tile that has an edge from
    # src. Rows are split by src HALF (local row </>= 1280) so each tile's
    # first gather call only depends on the first AllGather half.
    bin_of_pair = node_bin[u_dst]
    src_pos = pos[u_src]
    src_half = (src_pos % PAD_ROWS) // HALF
    src_hidx = (src_pos // PAD_ROWS) * HALF + (src_pos % PAD_ROWS) % HALF
    rk = (bin_of_pair * 2 + src_half) * FULL_HALF + src_hidx
    urow, row_inv = np.unique(rk, return_inverse=True)
    row_bh = urow // FULL_HALF
    row_psrc = urow % FULL_HALF          # index into the half table
    rows_per_bh = np.bincount(row_bh, minlength=n_bins * 2)
    k_a = int((rows_per_bh[0::2].max() + P - 1) // P)
    k_b = int((rows_per_bh[1::2].max() + P - 1) // P)
    k_max = k_a + k_b
    sizes = _gather_split(k_a) + _gather_split(k_b)

    # Rank within (bin, half) (urow sorted => grouped, ascending src pos),
    # then deal 16 ways so each SDMA engine (descriptor i -> engine i%16)
    # walks ascending HBM addresses.
    bh_starts = np.concatenate([[0], np.cumsum(rows_per_bh)])
    row_rank = np.arange(len(urow)) - bh_starts[row_bh]
    n_b = rows_per_bh[row_bh]
    sub_len = (n_b + 15) // 16
    pos_in_half = (row_rank % sub_len) * 16 + row_rank // sub_len
    row_pos = np.where(row_bh % 2 == 0, pos_in_half, k_a * P + pos_in_half)
    row_bin = row_bh // 2

    per_core = []
    pair_bin = bin_of_pair
    pair_rowpos = row_pos[row_inv]
    pair_slot = node_slot[u_dst]
    for c in range(N_CORES):
        idx_flat = np.zeros((TILES, k_max * P), np.int16)
        m = row_bin // TILES == c
        t = row_bin[m] % TILES
        idx_flat[t, row_pos[m]] = row_psrc[m].astype(np.int16)
        # S: [TILES*k_max*128 rows, 128 slots]
        S = np.zeros((TILES * k_max * P, P), F16)
        pm = pair_bin // TILES == c
        pt = pair_bin[pm] % TILES
        srow = pt * (k_max * P) + pair_rowpos[pm]
        S[srow, pair_slot[pm]] = mult[pm]
        per_core.append((idx_flat, S))

    # node at each padded position (for x shard + pooling construction)
    node_at = np.full(N_CORES * PAD_ROWS, -1, np.int64)
    node_at[pos] = np.arange(N_NODES)

    in_maps = []
    w1_pack = np.ascontiguousarray(
        W_g1.astype(F16).reshape(KCH, P, D).transpose(1, 0, 2).reshape(P, KCH * D))
    w2_pack = np.ascontiguousarray(
        W_g2.astype(F16).reshape(KCH, P, D).transpose(1, 0, 2).reshape(P, KCH * D))
    b1_pack = np.ascontiguousarray(np.broadcast_to(
        np.asarray(b_g1, np.float32), (P, D)))
    b2_pack = np.ascontiguousarray(np.broadcast_to(
        np.asarray(b_g2, np.float32), (P, D)))
    wc1_pack = np.ascontiguousarray(
        np.asarray(W_c1, np.float32).reshape(KCH, P, 2, P)
        .transpose(1, 0, 2, 3).reshape(P, KCH * 2 * P))
    bc1_pack = np.ascontiguousarray(np.asarray(b_c1, np.float32).reshape(2, P).T)
    wc2_pack = np.ascontiguousarray(np.asarray(W_c2, np.float32).reshape(2, P).T)
    bc2_pack = np.asarray(b_c2, np.float32).reshape(1, 1)

    x_bf = x.astype(F16)
    for c in range(N_CORES):
        idx_flat, S = per_core[c]

        # x shard in permuted position space
        nodes_c = node_at[c * PAD_ROWS : (c + 1) * PAD_ROWS]
        real = nodes_c >= 0
        xs = np.zeros((PAD_ROWS, D), F16)
        xs[real] = x_bf[nodes_c[real]]

        s_pack = np.ascontiguousarray(
            S.reshape(TILES * k_max, P, P).transpose(1, 0, 2).reshape(P, -1))

        # gather idx table [128, TILES*k_max*8] wrapped per call
        cols = []
        for t in range(TILES):
            off = 0
            for sz in sizes:
                block = idx_flat[t, off * P : (off + sz) * P]
                cols.append(_wrap_idx(block))
                off += sz
        idx16 = np.concatenate(cols, axis=1)          # [16, TILES*k_max*8]
        idx_pack = np.ascontiguousarray(np.tile(idx16, (8, 1)))

        # pooling one-hot [128, TILES*64]
        Pm = np.zeros((PAD_ROWS, N_GRAPHS), F16)
        Pm[real, batch[nodes_c[real]]] = 1
        p_pack = np.ascontiguousarray(
            Pm.reshape(TILES, P, N_GRAPHS).transpose(1, 0, 2).reshape(P, -1))

        in_maps.append({
            "x_sh": xs,
            "idx_all": idx_pack,
            "s_all": s_pack,
            "p_all": p_pack,
            "w1": w1_pack, "w2": w2_pack,
            "b1b": b1_pack, "b2b": b2_pack,
            "wc1": wc1_pack, "bc1": bc1_pack,
            "wc2": wc2_pack, "bc2": bc2_pack,
        })
    return (k_a, k_b), in_maps


def kernel(**inputs):
    global LAST_EXEC_NS, LAST_RESULTS
    kk, in_maps = _prep_inputs(**inputs)
    if kk not in _prog_cache:
        _prog_cache[kk] = _build_program(*kk)
    nc = _prog_cache[kk]
    trace = os.environ.get("GNN_TRACE", "0") == "1"
    res = run_bass_kernel_spmd(
        nc, in_maps, core_ids=list(range(N_CORES)), trace=trace,
        tmpdir=os.environ.get("GNN_TRACE_DIR") or None,
    )
    LAST_EXEC_NS = getattr(res, "exec_time_ns", None)
    LAST_RESULTS = res
    return np.asarray(res.results[0]["scores"]).reshape(N_GRAPHS).astype(np.float32)
